# revision 16
# baseline (speedup 1.0000x reference)
"""Trainium2 Bass kernel for late-interaction retrieval scoring (FLUKE+).

Math per doc n (see reference):
  sims[q,t] = q_emb[q] . doc[n,t]                       (late interaction)
  pts[q]    = soft-top3 aggregation of sims[q,:]        (softmax(top3/T).top3)
  base      = sum_q wq[q] pts[q]
  ASC: pmax = max_t sims; stats (mean/max/std/frac) -> MLP -> calib;
       asc = (sum_q wq pmax) * (1+tanh(...)); total = blend*base+(1-blend)*asc
  MGS: for k=1..3, k-gram mean-pooled renormalized doc embs, MaxSim:
       sims_k[q,t] = (sum_i sims[q,t+i]) / ||sum_i doc[t+i]||
       total += gw[k] * sum_q wq max_t sims_k
  TIR: total += relu(pts*qm @ w1 + b1) @ w2 + b2

Device mapping (per core, NDOC docs), v2:
  - doc embeddings host-transposed to [D=128, t] (bf16); contraction dim D
    on SBUF partitions. 4 docs per PSUM sims tile via col-tiled matmuls
    (M=32 each, tile_position=(0,32b)).
  - k-gram numerators num2/num3 = shifted adds of sims, done on GPSIMD
    (alignment-immune) from an ACT-copied bf16 sims tile.
  - pairwise grams pp1/pp2 on DVE (2x mode); D-reduction via
    one-hot-column ones matmuls (one 358-col MM per doc);
    3-gram norm combined on GPSIMD; rsqrt = exp(-0.5*ln(2x+{2,3})) on ACT
    at doc resolution.
  - inv broadcast to all 32 q-rows: ONE [128x128] selector matmul per
    group (357 cols) instead of per-doc broadcasts.
  - scr_k = num_k * inv_k (one DVE mult per 2 groups, invb read straight
    from PSUM across 2 banks) + one segmented reduce_max into redc[...,2].
  - top-3 via DVE max8 on the bf16 sims copy.
  - finishing phase (per core, once): softmax over top3, weighted stats via
    col+row-tiled matmuls, ASC/TIR MLPs, combine, all on [*,128] tiles.
"""

import os
import numpy as np

# ---- problem constants (hardcoded; kernel.py must be self-contained)
N, NQ, ND, D = 4096, 32, 180, 128
TOPK, TEMP, MAXK = 3, 0.1, 3
NCORES = 8
NDOC = N // NCORES            # 512 docs per core
GROUP = 4                     # docs packed per col-tiled psum tile
SUPER = 32                    # docs per supergroup
NW2, NW3 = ND - 1, ND - 2     # 179, 178
NW = NW2 + NW3                # 357
BF = np.float16

_CACHE = {}


def _build_program(ndoc):
    """Build the SPMD Bass program for one core processing `ndoc` docs."""
    import concourse.bass as bass
    import concourse.tile as tile
    from concourse import mybir
    from contextlib import ExitStack

    f32, bf16 = mybir.dt.float32, mybir.dt.float16
    AF = mybir.ActivationFunctionType
    OP = mybir.AluOpType

    NSG = ndoc // SUPER       # supergroups
    NG = ndoc // GROUP        # total groups (= finishing tile width)
    GPS = SUPER // GROUP      # groups per supergroup = 8
    use_gps = os.environ.get("NO_GPSIMD", "0") != "1"
    use_ttr = os.environ.get("NO_TTR", "0") != "1"

    nc = bass.Bass()

    # ---------------- DRAM I/O ----------------
    docT = nc.dram_tensor("docT", [128, ndoc * ND], bf16, kind="ExternalInput")
    qT_d = nc.dram_tensor("qT", [128, NQ], bf16, kind="ExternalInput")
    selg_d = nc.dram_tensor("selg", [128, 32 * GPS], bf16, kind="ExternalInput")
    sful_d = nc.dram_tensor("sful", [128, 128 * GPS], bf16, kind="ExternalInput")
    wq32_d = nc.dram_tensor("wq32", [128, 32], f32, kind="ExternalInput")
    qmn32_d = nc.dram_tensor("qmn32", [128, 32], f32, kind="ExternalInput")
    tirw1_d = nc.dram_tensor("tirw1", [128, 64], f32, kind="ExternalInput")
    tirw2_d = nc.dram_tensor("tirw2", [128, 32], f32, kind="ExternalInput")
    tirb1_d = nc.dram_tensor("tirb1", [128, 1], f32, kind="ExternalInput")
    ascw1_d = nc.dram_tensor("ascw1", [128, 96], f32, kind="ExternalInput")
    ascw2_d = nc.dram_tensor("ascw2", [128, 32], f32, kind="ExternalInput")
    ascb1_d = nc.dram_tensor("ascb1", [128, 1], f32, kind="ExternalInput")
    ascb2x2_d = nc.dram_tensor("ascb2x2", [128, 1], f32, kind="ExternalInput")
    tirb2_d = nc.dram_tensor("tirb2", [128, 1], f32, kind="ExternalInput")
    qm4_d = nc.dram_tensor("qm4", [128, 1], f32, kind="ExternalInput")
    params_d = nc.dram_tensor("params", [1, 8], f32, kind="ExternalInput")
    out_d = nc.dram_tensor("out", [GROUP, NG], f32, kind="ExternalOutput")

    docT_v = docT.rearrange("p (d t) -> p d t", t=ND)

    with ExitStack() as ctx:
        tc = ctx.enter_context(tile.TileContext(nc))
        const = ctx.enter_context(tc.tile_pool(name="const", bufs=1))
        dpool = ctx.enter_context(tc.tile_pool(name="dpool", bufs=3))
        ppool = ctx.enter_context(tc.tile_pool(name="ppool", bufs=2))
        gpool = ctx.enter_context(tc.tile_pool(name="gpool", bufs=3))
        work = ctx.enter_context(tc.tile_pool(name="work", bufs=6))
        big = ctx.enter_context(tc.tile_pool(name="big", bufs=2))
        coll = ctx.enter_context(tc.tile_pool(name="coll", bufs=1))
        fin = ctx.enter_context(tc.tile_pool(name="fin", bufs=1))
        mainps = ctx.enter_context(ExitStack())
        ps = mainps.enter_context(tc.tile_pool(name="ps", bufs=3, space="PSUM"))
        psb = mainps.enter_context(tc.tile_pool(name="psb", bufs=2, space="PSUM"))
        psg = mainps.enter_context(tc.tile_pool(name="psg", bufs=1, space="PSUM"))

        # ---------------- constants ----------------
        qT = const.tile([128, NQ], bf16)
        nc.sync.dma_start(out=qT, in_=qT_d[:])
        selg = const.tile([128, 32 * GPS], bf16)
        nc.sync.dma_start(out=selg, in_=selg_d[:])
        sful = const.tile([128, 128 * GPS], bf16)
        nc.sync.dma_start(out=sful, in_=sful_d[:])
        wq32 = const.tile([128, 32], f32)
        nc.sync.dma_start(out=wq32, in_=wq32_d[:])
        qmn32 = const.tile([128, 32], f32)
        nc.sync.dma_start(out=qmn32, in_=qmn32_d[:])
        tirw1 = const.tile([128, 64], f32)
        nc.sync.dma_start(out=tirw1, in_=tirw1_d[:])
        tirw2 = const.tile([128, 32], f32)
        nc.sync.dma_start(out=tirw2, in_=tirw2_d[:])
        tirb1 = const.tile([128, 1], f32)
        nc.sync.dma_start(out=tirb1, in_=tirb1_d[:])
        ascw1 = const.tile([128, 96], f32)
        nc.sync.dma_start(out=ascw1, in_=ascw1_d[:])
        ascw2 = const.tile([128, 32], f32)
        nc.sync.dma_start(out=ascw2, in_=ascw2_d[:])
        ascb1 = const.tile([128, 1], f32)
        nc.sync.dma_start(out=ascb1, in_=ascb1_d[:])
        ascb2x2 = const.tile([128, 1], f32)
        nc.sync.dma_start(out=ascb2x2, in_=ascb2x2_d[:])
        tirb2 = const.tile([128, 1], f32)
        nc.sync.dma_start(out=tirb2, in_=tirb2_d[:])
        qm4 = const.tile([128, 1], f32)
        nc.sync.dma_start(out=qm4, in_=qm4_d[:])
        pvec = const.tile([1, 8], f32)
        nc.sync.dma_start(out=pvec, in_=params_d[:])

        b_zero = const.tile([128, 1], f32)
        nc.vector.memset(b_zero, 0.0)
        b_two = const.tile([128, 1], f32)
        nc.vector.memset(b_two, 2.0)
        b_three = const.tile([128, 1], f32)
        nc.vector.memset(b_three, 3.0)
        b_eps = const.tile([128, 1], f32)
        nc.vector.memset(b_eps, 1e-6)
        ones_row = const.tile([1, 128], f32)
        nc.vector.memset(ones_row, 1.0)

        # ---------------- collectors ----------------
        top8c = coll.tile([128, NG, 8], f32)
        redc = coll.tile([128, NG, 2], f32)

        # ---------------- main loop ----------------
        for sg in range(NSG):
            d0 = sg * SUPER
            dt_ = dpool.tile([128, SUPER, ND], bf16, tag="dt")
            nc.gpsimd.dma_start(out=dt_, in_=docT_v[:, d0:d0 + SUPER, :])

            # pairwise grams; pp layout [128, d, 360]: pp1@[0:179] pp2@[180:358]
            pp = ppool.tile([128, SUPER, 360], bf16, tag="pp")
            nc.vector.tensor_mul(pp[:, :, 0:NW2],
                                 dt_[:, :, 0:NW2], dt_[:, :, 1:ND])
            nc.vector.tensor_mul(pp[:, :, 180:180 + NW3],
                                 dt_[:, :, 0:NW3], dt_[:, :, 2:ND])
            nc.vector.memset(pp[:, :, NW2:180], 0.0)

            # D-reduction: gsum[32b+j, 0:179]=g1, [180:358]=g2 for doc 4j+b
            gsum_t = psg.tile([128, 512], f32, tag="gsum")
            gsum = gsum_t[:, 0:358]
            for dd in range(SUPER):
                j, b = dd // GROUP, dd % GROUP
                sel = selg[:, 32 * j:32 * (j + 1)]
                nc.tensor.matmul(gsum[32 * b:32 * (b + 1), :], sel,
                                 pp[:, dd, 0:358], start=True, stop=True,
                                 tile_position=(0, 32 * b),
                                 skip_group_check=True)

            # norms -> inv = rsqrt at doc resolution
            # inv layout (184-aligned regions): R2@[0:184] R3@[184:368]
            g_sb = gpool.tile([128, 358], bf16, tag="gsb")
            nc.scalar.copy(g_sb, gsum)
            t3 = gpool.tile([128, NW3], bf16, tag="t3")
            nc.vector.tensor_add(t3, g_sb[:, 0:NW3], g_sb[:, 1:NW2])
            n3in = gpool.tile([128, NW3], bf16, tag="n3in")
            nc.vector.tensor_add(n3in, t3, g_sb[:, 180:358])
            lnn = gpool.tile([128, 388], f32, tag="lnn")
            nc.scalar.activation(out=lnn[:, 0:NW2], in_=g_sb[:, 0:NW2],
                                 func=AF.Ln, bias=b_two, scale=2.0)
            nc.scalar.activation(out=lnn[:, 204:204 + NW3], in_=n3in,
                                 func=AF.Ln, bias=b_three, scale=2.0)
            inv = gpool.tile([128, 408], bf16, tag="inv")
            nc.vector.memset(inv[:, NW2:204], 0.0)
            nc.vector.memset(inv[:, 204 + NW3:408], 0.0)
            nc.scalar.activation(out=inv[:, 0:NW2], in_=lnn[:, 0:NW2],
                                 func=AF.Exp, bias=b_zero, scale=-0.5)
            nc.scalar.activation(out=inv[:, 204:204 + NW3],
                                 in_=lnn[:, 204:204 + NW3],
                                 func=AF.Exp, bias=b_zero, scale=-0.5)

            for j in range(GPS):
                gg = GPS * sg + j
                p2 = j % 2
                if j == 0:
                    # supergroup-wide sims/num and inv tiles (fp16)
                    sev = big.tile([128, 8, 592], bf16, tag="sev")
                    nc.vector.memset(sev[:, :, 359:364], -60000.0)
                    nc.vector.memset(sev[:, :, 562:568], -60000.0)
                    invb_sb = big.tile([128, 8, 408], bf16, tag="invsb")
                if p2 == 0:
                    # invb spans 2 PSUM banks per 2 groups
                    invb2g_t = psb.tile([128, 1024], f32, tag="invb")
                # sims@[0:180] and num2@[180:359] accumulate in one bank
                ps_sims_t = ps.tile([128, 512], f32, tag="sims")
                ps_sims = ps_sims_t[:, 0:ND]
                ps_num2 = ps_sims_t[:, ND:ND + NW2]
                for b in range(GROUP):
                    d = GROUP * j + b
                    rows = slice(32 * b, 32 * (b + 1))
                    tp = (0, 32 * b)
                    nc.tensor.matmul(ps_sims[rows, :], qT,
                                     dt_[:, d, :], start=True, stop=True,
                                     tile_position=tp, skip_group_check=True)
                    nc.tensor.matmul(ps_num2[rows, :], qT,
                                     dt_[:, d, 0:NW2], start=True, stop=False,
                                     tile_position=tp, skip_group_check=True)
                    nc.tensor.matmul(ps_num2[rows, :], qT,
                                     dt_[:, d, 1:ND], start=False, stop=True,
                                     tile_position=tp, skip_group_check=True)
                # inv rows {32b+j} broadcast to all 32 q-rows, one matmul
                nc.tensor.matmul(invb2g_t[:, 512 * p2:512 * p2 + 408],
                                 sful[:, 128 * j:128 * (j + 1)],
                                 inv, start=True, stop=True,
                                 tile_position=(0, 0), skip_group_check=True)
                if p2 == 1:
                    # both halves written: move invb to SBUF fp16 (ACT)
                    nc.scalar.copy(
                        invb_sb[:, j - 1:j + 1, :],
                        invb2g_t.rearrange("p (g c) -> p g c",
                                           c=512)[:, :, 0:408])

                # sims+num2 -> SBUF fp16 in one ACT copy
                nc.scalar.copy(sev[:, j, 0:359], ps_sims_t[:, 0:359])
                nc.vector.max(top8c[:, gg, :], sev[:, j, 0:ND])

                if j == GPS - 1:
                    # num3 = num2 + s[t+2], one aligned 2x add for 8 groups
                    nc.vector.tensor_add(sev[:, :, 384:384 + NW3],
                                         sev[:, :, 180:180 + NW3],
                                         sev[:, :, 2:ND])
                    # scr = num*inv (all-SBUF fp16) + segmented max, 8 grp
                    scr8 = big.tile([128, 8, 368], bf16, tag="scr")
                    num_v = sev[:, :, 180:588].rearrange(
                        "p g (r c) -> p g r c", c=204)[:, :, :, 0:184]
                    invs_v = invb_sb.rearrange(
                        "p g (r c) -> p g r c", c=204)[:, :, :, 0:184]
                    nc.vector.tensor_mul(
                        scr8.rearrange("p g (r c) -> p g r c", c=184),
                        num_v, invs_v)
                    nc.vector.reduce_max(
                        redc[:, gg - 7:gg + 1, :],
                        scr8.rearrange("p g (r c) -> p g r c", c=184),
                        axis=mybir.AxisListType.X)

        # ---------------- finishing phase ----------------
        mainps.close()
        ps = ctx.enter_context(tc.tile_pool(name="psf", bufs=1, space="PSUM"))

        topv = top8c[:, :, 0:TOPK]              # [128, NG, 3]
        pmax = top8c[:, :, 0:1]                 # [128, NG, 1]

        # pts = softmax(topv/T).topv
        exps = fin.tile([128, NG, TOPK], f32)
        nc.scalar.activation(out=exps, in_=topv, func=AF.Exp,
                             bias=b_zero, scale=1.0 / TEMP)
        den = fin.tile([128, NG], f32)
        nc.vector.reduce_sum(den, exps, axis=mybir.AxisListType.X)
        wnum = fin.tile([128, NG, TOPK], f32)
        nc.vector.tensor_mul(wnum, exps, topv)
        pnum = fin.tile([128, NG], f32)
        nc.vector.reduce_sum(pnum, wnum, axis=mybir.AxisListType.X)
        rden = fin.tile([128, NG], f32)
        nc.vector.reciprocal(rden, den)
        pts = fin.tile([128, NG], f32)
        nc.vector.tensor_mul(pts, pnum, rden)
        # padded = pts * qm (qm broadcast per partition block)
        pts_t = fin.tile([128, NG], f32)
        nc.vector.tensor_scalar(out=pts_t, in0=pts, scalar1=qm4, scalar2=None,
                                op0=OP.mult)
        pmax2 = fin.tile([128, NG], f32)
        nc.vector.tensor_mul(pmax2, pmax[:, :, 0], pmax[:, :, 0])

        # stats matmuls, col+row tiled per strip: all outputs on rows {32b}
        ps_stat_t = ps.tile([128, 512], f32, tag="stat")
        ps_stat = ps_stat_t[:, 0:3 * NG]        # asc_base | mean | msq
        ps_stat2_t = ps.tile([128, 512], f32, tag="stat2")
        ps_stat2 = ps_stat2_t[:, 0:3 * NG]      # red2w | red3w | base
        ps_mlp_t = ps.tile([128, 512], f32, tag="mlp")
        ps_mlp = ps_mlp_t[:, 0:NG]              # H
        ps_base = ps_stat2[:, 2 * NG:3 * NG]
        pmax_2d = top8c[:, :, 0]                # [128, NG] strided view
        for b in range(GROUP):
            rows = slice(32 * b, 32 * (b + 1))
            tp = (32 * b, 32 * b)
            nc.tensor.matmul(ps_stat[rows, 0:NG], wq32[rows, :],
                             pmax_2d[rows, :], start=True, stop=True,
                             tile_position=tp, skip_group_check=True)
            nc.tensor.matmul(ps_stat[rows, NG:2 * NG], qmn32[rows, :],
                             pmax_2d[rows, :], start=True, stop=True,
                             tile_position=tp, skip_group_check=True)
            nc.tensor.matmul(ps_stat[rows, 2 * NG:3 * NG], qmn32[rows, :],
                             pmax2[rows, :], start=True, stop=True,
                             tile_position=tp, skip_group_check=True)
            nc.tensor.matmul(ps_base[rows, :], wq32[rows, :],
                             pts[rows, :], start=True, stop=True,
                             tile_position=tp, skip_group_check=True)
            nc.tensor.matmul(ps_stat2[rows, 0:NG], wq32[rows, :],
                             redc[rows, :, 0], start=True, stop=True,
                             tile_position=tp, skip_group_check=True)
            nc.tensor.matmul(ps_stat2[rows, NG:2 * NG], wq32[rows, :],
                             redc[rows, :, 1], start=True, stop=True,
                             tile_position=tp, skip_group_check=True)

        # single ordered whole-bank reads (PSUM banks must not be read while
        # PE still writes other columns of the same bank)
        stat_sb = fin.tile([128, 3 * NG], f32)
        nc.scalar.copy(stat_sb, ps_stat)
        stat2_sb = fin.tile([128, 3 * NG], f32)
        nc.scalar.copy(stat2_sb, ps_stat2)
        # mx: max over q (transpose + free-dim reduce + transpose back)
        ident = const.tile([128, 128], f32)
        nc.sync.dma_start(out=ident, in_=nc.dram_tensor(
            "ident", [128, 128], f32, kind="ExternalInput")[:])
        pmax_sb = fin.tile([128, NG], f32)
        nc.vector.tensor_copy(pmax_sb, pmax_2d)
        ps_pT_t = ps.tile([128, 512], f32, tag="ptrans")
        ps_pT = ps_pT_t[:, 0:128]
        nc.tensor.transpose(ps_pT[0:NG, :], pmax_sb, ident)
        mxT = fin.tile([128, GROUP], f32)
        nc.vector.reduce_max(mxT[0:NG, :], ps_pT[0:NG, :].rearrange(
            "g (b q) -> g b q", q=32), axis=mybir.AxisListType.X)
        mxpad = fin.tile([128, 128], f32)
        nc.vector.memset(mxpad, 0.0)
        nc.vector.tensor_copy(
            mxpad[0:NG, :].rearrange("g (b o) -> g b o", o=32)[:, :, 0:1],
            mxT[0:NG, :].rearrange("g (b o) -> g b o", o=1))
        ps_mxb_t = ps.tile([128, 512], f32, tag="ptrans")
        ps_mxb = ps_mxb_t[:, 0:128]
        nc.tensor.transpose(ps_mxb[:, 0:NG], mxpad[0:NG, :], ident[0:NG, 0:NG])

        # ---- per-strip finishing: every accessed row is at base 32b ----
        F = fin.tile([128, 3 * NG], f32)    # cols: mean | mx | std, rows {32b}
        scrA = fin.tile([128, NG], f32)
        scrB = fin.tile([128, NG], f32)
        for b in range(GROUP):
            row = slice(32 * b, 32 * b + 1)
            mean = stat_sb[row, NG:2 * NG]
            msq = stat_sb[row, 2 * NG:3 * NG]
            nc.vector.tensor_copy(F[row, 0:NG], mean)
            nc.scalar.copy(F[row, NG:2 * NG], ps_mxb[row, 0:NG])
            # var = msq - mean^2 -> std = exp(0.5*ln(var + 1e-6))
            nc.vector.tensor_mul(scrA[row, :], F[row, 0:NG], F[row, 0:NG])
            nc.vector.tensor_sub(scrB[row, :], msq, scrA[row, :])
            nc.scalar.activation(out=scrA[row, :], in_=scrB[row, :],
                                 func=AF.Ln, bias=b_eps[row, :], scale=1.0)
            nc.scalar.activation(out=F[row, 2 * NG:3 * NG], in_=scrA[row, :],
                                 func=AF.Exp, bias=b_zero[row, :], scale=0.5)

        # ASC MLP via K=1 accumulation over the 3 features
        for b in range(GROUP):
            row = slice(32 * b, 32 * b + 1)
            for s in range(3):
                nc.tensor.matmul(ps_mlp[32 * b:32 * (b + 1), :],
                                 ascw1[row, 32 * s:32 * (s + 1)],
                                 F[row, s * NG:(s + 1) * NG],
                                 start=(s == 0), stop=(s == 2),
                                 tile_position=(32 * b, 32 * b),
                                 skip_group_check=True)
        Hs = fin.tile([128, NG], f32)
        nc.scalar.activation(out=Hs, in_=ps_mlp, func=AF.Relu, bias=ascb1, scale=1.0)
        ps_calsc_t = ps.tile([128, 512], f32, tag="cal")
        ps_cal = ps_calsc_t[:, 0:NG]
        ps_sc = ps_calsc_t[:, NG:NG + 8]
        for b in range(GROUP):
            nc.tensor.matmul(ps_cal[32 * b:32 * (b + 1), :],
                             ascw2[32 * b:32 * (b + 1), :],
                             Hs[32 * b:32 * (b + 1), :], start=True, stop=True,
                             tile_position=(32 * b, 32 * b), skip_group_check=True)
        # gw = softmax(mgs_logits); blend = sigmoid(asc_blend)
        ge = fin.tile([1, 3], f32)
        gden = fin.tile([1, 1], f32)
        nc.scalar.activation(out=ge, in_=pvec[0:1, 0:3], func=AF.Exp,
                             bias=b_zero[0:1, :], scale=1.0)
        nc.vector.reduce_sum(gden, ge, axis=mybir.AxisListType.X)
        grden = fin.tile([1, 1], f32)
        nc.vector.reciprocal(grden, gden)
        svec = fin.tile([1, 8], f32)
        nc.vector.memset(svec, 0.0)
        nc.vector.tensor_scalar(out=svec[0:1, 0:3], in0=ge, scalar1=grden,
                                scalar2=None, op0=OP.mult)
        be = fin.tile([1, 1], f32)
        nc.scalar.activation(out=be, in_=pvec[0:1, 3:4], func=AF.Exp,
                             bias=b_zero[0:1, :], scale=-1.0)
        nc.vector.tensor_scalar(out=be, in0=be, scalar1=1.0, scalar2=None, op0=OP.add)
        nc.vector.reciprocal(svec[0:1, 3:4], be)                       # blend
        nc.vector.tensor_scalar(out=svec[0:1, 4:5], in0=svec[0:1, 3:4],
                                scalar1=-1.0, scalar2=1.0, op0=OP.mult, op1=OP.add)
        nc.tensor.matmul(ps_sc, ones_row, svec, start=True, stop=True,
                         tile_position=(0, 0), skip_group_check=True)
        calsc_sb = fin.tile([128, NG + 8], f32)
        nc.scalar.copy(calsc_sb, ps_calsc_t[:, 0:NG + 8])
        sc = calsc_sb[:, NG:NG + 8]
        cal_sb = calsc_sb[:, 0:NG]

        # TIR MLP (row-tiled K=32); A/B in separate banks (PE-W vs ACT-R hazard)
        ps_tirA_t = ps.tile([128, 512], f32, tag="tirA")
        ps_tirA = ps_tirA_t[:, 0:NG]
        ps_tirB_t = ps.tile([128, 512], f32, tag="tirB")
        ps_tirB = ps_tirB_t[:, 0:NG]
        tir_dst = [(ps_tirA, 0), (ps_tirA, 64), (ps_tirB, 0), (ps_tirB, 64)]
        for b in range(GROUP):
            dst, o = tir_dst[b]
            nc.tensor.matmul(dst[o:o + 64, :], tirw1[32 * b:32 * (b + 1), :],
                             pts_t[32 * b:32 * (b + 1), :], start=True, stop=True,
                             tile_position=(32 * b, o), skip_group_check=True)
        HsA = fin.tile([128, NG], f32)
        HsB = fin.tile([128, NG], f32)
        nc.scalar.activation(out=HsA, in_=ps_tirA, func=AF.Relu, bias=tirb1, scale=1.0)
        nc.scalar.activation(out=HsB, in_=ps_tirB, func=AF.Relu, bias=tirb1, scale=1.0)
        ps_tv_t = ps.tile([128, 512], f32, tag="tv")
        ps_tv = ps_tv_t[:, 0:NG]
        tir_src = [(HsA, 0), (HsA, 64), (HsB, 0), (HsB, 64)]
        for b in range(GROUP):
            src_t, o = tir_src[b]
            nc.tensor.matmul(ps_tv[32 * b:32 * (b + 1), :], tirw2[o:o + 64, :],
                             src_t[o:o + 64, :], start=True, stop=True,
                             tile_position=(o, 32 * b), skip_group_check=True)

        tv_sb = fin.tile([128, NG], f32)
        nc.scalar.copy(tv_sb, ps_tv)
        # combine per strip on row {32b}:
        # total = blend*base + (1-blend)*asc_base*(1+calib)
        #         + gw0*asc_base + gw1*red2w + gw2*red3w + tirv + tir_b2
        tot = fin.tile([128, NG], f32)
        for b in range(GROUP):
            row = slice(32 * b, 32 * b + 1)
            asc_base = stat_sb[row, 0:NG]
            base = stat2_sb[row, 2 * NG:3 * NG]
            # calib1 = 1 + tanh(x + b2) = 2 - 2/(exp(2x+2b2)+1)
            nc.scalar.activation(out=scrA[row, :], in_=cal_sb[row, :],
                                 func=AF.Exp, bias=ascb2x2[row, :], scale=2.0)
            nc.vector.tensor_scalar(out=scrA[row, :], in0=scrA[row, :],
                                    scalar1=1.0, scalar2=None, op0=OP.add)
            nc.vector.reciprocal(scrB[row, :], scrA[row, :])
            nc.vector.tensor_scalar(out=scrB[row, :], in0=scrB[row, :],
                                    scalar1=-2.0, scalar2=2.0, op0=OP.mult, op1=OP.add)
            nc.vector.tensor_mul(scrB[row, :], scrB[row, :], asc_base)
            nc.vector.tensor_scalar(out=scrB[row, :], in0=scrB[row, :],
                                    scalar1=sc[row, 4:5], scalar2=None, op0=OP.mult)
            nc.vector.tensor_scalar(out=scrA[row, :], in0=base,
                                    scalar1=sc[row, 3:4], scalar2=None, op0=OP.mult)
            nc.vector.tensor_add(tot[row, :], scrA[row, :], scrB[row, :])
            nc.vector.tensor_scalar(out=scrA[row, :], in0=asc_base,
                                    scalar1=sc[row, 0:1], scalar2=None, op0=OP.mult)
            nc.vector.tensor_add(tot[row, :], tot[row, :], scrA[row, :])
            nc.vector.tensor_scalar(out=scrA[row, :], in0=stat2_sb[row, 0:NG],
                                    scalar1=sc[row, 1:2], scalar2=None, op0=OP.mult)
            nc.vector.tensor_add(tot[row, :], tot[row, :], scrA[row, :])
            nc.vector.tensor_scalar(out=scrA[row, :], in0=stat2_sb[row, NG:2 * NG],
                                    scalar1=sc[row, 2:3], scalar2=None, op0=OP.mult)
            nc.vector.tensor_add(tot[row, :], tot[row, :], scrA[row, :])
            nc.vector.tensor_add(tot[row, :], tot[row, :], tv_sb[row, :])
            nc.vector.tensor_scalar(out=tot[row, :], in0=tot[row, :],
                                    scalar1=tirb2[row, :], scalar2=None, op0=OP.add)
            nc.sync.dma_start(out=out_d[b:b + 1, :], in_=tot[row, :])

    return nc


def _legalize_single_wait(nc):
    """Walrus (this compile path) accepts at most one sync wait per
    instruction; offload extra waits onto preceding EventSemaphore
    instructions on the same engine queue."""
    from concourse import mybir
    ctr = [0]
    for bb in nc.main_func.blocks:
        il = bb.instructions
        out = []
        for inst in il:
            si = inst.sync_info
            if si is not None and len(si.on_wait) > 1:
                waits = list(si.on_wait)
                eng = nc.engines[inst.engine]
                for w in waits[:-1]:
                    ev = eng._isa(
                        nc.isa.Opcode.NEURON_ISA_TPB_OPCODE_NOP, {})
                    ev.sync_info = mybir.SyncInfo(on_wait=[w], on_update=[])
                    ctr[0] += 1
                    try:
                        nc.register_instruction(ev)
                    except Exception:
                        pass
                    out.append(ev)
                inst.sync_info = mybir.SyncInfo(on_wait=[waits[-1]],
                                                on_update=list(si.on_update))
            out.append(inst)
        bb.instructions = out
    return nc


def _host_prep(inputs, ndoc_per_core, ncores):
    """Shard + lay out inputs for the SPMD program. Returns list of in_maps."""
    q = np.asarray(inputs["query_embs"], np.float32)          # [NQ, D]
    docs = np.asarray(inputs["doc_embs"], np.float32)         # [N, ND, D]
    w = np.asarray(inputs["importance_weights"], np.float32)  # [NQ]
    qm = np.asarray(inputs["query_mask"]).astype(np.float32)  # [NQ]
    asc_w1 = np.asarray(inputs["asc_w1"], np.float32)
    asc_b1 = np.asarray(inputs["asc_b1"], np.float32)
    asc_w2 = np.asarray(inputs["asc_w2"], np.float32)
    asc_b2 = np.float32(inputs["asc_b2"])
    asc_blend = np.float32(inputs["asc_blend"])
    mgs_logits = np.asarray(inputs["mgs_logits"], np.float32)
    tir_w1 = np.asarray(inputs["tir_w1"], np.float32)
    tir_b1 = np.asarray(inputs["tir_b1"], np.float32)
    tir_w2 = np.asarray(inputs["tir_w2"], np.float32)
    tir_b2 = np.float32(inputs["tir_b2"])

    wq = (w * qm).astype(np.float32)
    nvalid = float(qm.sum())
    frac = nvalid / NQ

    # frac folded into ASC layer-1 bias; feats order = [mean, mx, std]
    b1p = asc_b1 + frac * asc_w1[3, :]
    w1p = asc_w1[:3, :]                                       # [3, 32]

    GPS = SUPER // GROUP
    qT = np.ascontiguousarray(q.T).astype(BF)                 # [128, 32]
    selg = np.zeros((128, 32 * GPS), BF)
    for j in range(GPS):
        selg[:, 32 * j + j] = 1
    # sful: S_j[32b+j, 128j + 32b+q] = 1 (bcast inv row 32b+j -> rows 32b+q)
    sful = np.zeros((128, 128 * GPS), BF)
    for j in range(GPS):
        for b in range(GROUP):
            sful[32 * b + j, 128 * j + 32 * b:128 * j + 32 * (b + 1)] = 1
    wq32 = np.zeros((128, 32), np.float32)
    wq32[:, 0] = np.tile(wq, 4)
    qmn32 = np.zeros((128, 32), np.float32)
    qmn32[:, 0] = np.tile(qm / max(nvalid, 1e-9), 4)
    tirw1 = np.tile(tir_w1, (4, 1)).astype(np.float32)        # [128, 64]
    tirw2 = np.zeros((128, 32), np.float32)
    tirw2[:, 0] = np.tile(tir_w2, 2)
    tirb1 = np.tile(tir_b1, 2).reshape(128, 1).astype(np.float32)
    ascw1 = np.zeros((128, 96), np.float32)
    for b in range(4):
        for s in range(3):
            ascw1[32 * b, 32 * s:32 * (s + 1)] = w1p[s, :]
    ascw2 = np.zeros((128, 32), np.float32)
    ascw2[:, 0] = np.tile(asc_w2, 4)
    ascb1 = np.tile(b1p, 4).reshape(128, 1).astype(np.float32)
    ascb2x2 = np.full((128, 1), 2.0 * asc_b2, np.float32)
    tirb2 = np.full((128, 1), tir_b2, np.float32)
    qm4 = np.tile(qm, 4).reshape(128, 1).astype(np.float32)
    params = np.zeros((1, 8), np.float32)
    params[0, 0:3] = mgs_logits
    params[0, 3] = asc_blend
    ident = np.eye(128, dtype=np.float32)

    shared = dict(qT=qT, selg=selg, sful=sful, wq32=wq32, qmn32=qmn32,
                  tirw1=tirw1, tirw2=tirw2, tirb1=tirb1, ascw1=ascw1,
                  ascw2=ascw2, ascb1=ascb1, ascb2x2=ascb2x2, tirb2=tirb2,
                  qm4=qm4, params=params, ident=ident)

    in_maps = []
    for c in range(ncores):
        sl = docs[c * ndoc_per_core:(c + 1) * ndoc_per_core]   # [ndoc, ND, D]
        dT = np.ascontiguousarray(sl.transpose(2, 0, 1)).astype(BF)
        m = dict(shared)
        m["docT"] = dT.reshape(128, ndoc_per_core * ND)
        in_maps.append(m)
    return in_maps


def _numpy_fallback(inputs):
    """Full-precision numpy implementation (only for non-all-ones masks)."""
    q = np.asarray(inputs["query_embs"], np.float64)
    docs = np.asarray(inputs["doc_embs"], np.float64)
    w = np.asarray(inputs["importance_weights"], np.float64)
    qm_b = np.asarray(inputs["query_mask"]).astype(bool)
    dm_b = np.asarray(inputs["doc_mask"]).astype(bool)
    NEG = -1e9
    qm = qm_b.astype(np.float64)
    wq = w * qm
    sims = np.einsum("qd,ntd->nqt", q, docs)
    sims = np.where(dm_b[:, None, :], sims, NEG)
    topv = -np.sort(-sims, axis=-1)[:, :, :TOPK]
    e = np.exp((topv - topv.max(-1, keepdims=True)) / TEMP)
    soft = e / e.sum(-1, keepdims=True)
    pts = (soft * topv).sum(-1)
    base = pts @ wq
    pmax = sims.max(-1)
    asc_base = pmax @ wq
    nvalid = qm.sum()
    mean = (pmax * qm).sum(-1) / nvalid
    mx = np.where(qm_b, pmax, NEG).max(-1)
    std = np.sqrt((((pmax - mean[:, None]) ** 2) * qm).sum(-1) / nvalid + 1e-6)
    frac = np.full_like(mean, nvalid / NQ)
    feats = np.stack([mean, mx, std, frac], -1)
    h = np.maximum(feats @ np.asarray(inputs["asc_w1"], np.float64)
                   + np.asarray(inputs["asc_b1"], np.float64), 0)
    calib = np.tanh(h @ np.asarray(inputs["asc_w2"], np.float64)
                    + float(inputs["asc_b2"]))
    asc_score = asc_base * (1.0 + calib)
    blend = 1 / (1 + np.exp(-float(inputs["asc_blend"])))
    total = blend * base + (1 - blend) * asc_score
    gl = np.asarray(inputs["mgs_logits"], np.float64)
    gw = np.exp(gl - gl.max()); gw /= gw.sum()
    dmf = dm_b.astype(np.float64)
    for k in range(1, MAXK + 1):
        if k == 1:
            dk, mk = docs, dm_b
        else:
            nw = ND - k + 1
            s = sum(docs[:, i:i + nw] for i in range(k)) / k
            dk = s / np.sqrt((s * s).sum(-1, keepdims=True) + 1e-12)
            mkf = dmf[:, 0:nw].copy()
            for i in range(1, k):
                mkf = mkf * dmf[:, i:i + nw]
            mk = mkf > 0.5
        sk = np.einsum("qd,nwd->nqw", q, dk)
        sk = np.where(mk[:, None, :], sk, NEG)
        total = total + gw[k - 1] * (sk.max(-1) @ wq)
    padded = pts * qm
    hres = np.maximum(padded @ np.asarray(inputs["tir_w1"], np.float64)
                      + np.asarray(inputs["tir_b1"], np.float64), 0)
    total = total + hres @ np.asarray(inputs["tir_w2"], np.float64) + float(inputs["tir_b2"])
    return total.astype(np.float32)


def kernel(**inputs):
    qm = np.asarray(inputs["query_mask"]).astype(bool)
    dm = np.asarray(inputs["doc_mask"]).astype(bool)
    if not (qm.all() and dm.all()):
        return _numpy_fallback(inputs)

    from concourse.bass_utils import run_bass_kernel_spmd

    key = ("prog", NDOC)
    if key not in _CACHE:
        from concourse import mybir
        nc = _legalize_single_wait(_build_program(NDOC))
        # populate .instr bytes for extended-inst ISA subclasses
        # (tensor_tensor_reduce); without this walrus fails with
        # "ISA wrong length".
        mybir.codegen_inst_isa_subclasses(nc)
        _CACHE[key] = nc
    nc = _CACHE[key]

    in_maps = _host_prep(inputs, NDOC, NCORES)
    res = run_bass_kernel_spmd(nc, in_maps, list(range(NCORES)))
    out = np.empty((N,), np.float32)
    for c in range(NCORES):
        grid = res.results[c]["out"]            # [4, NG] -> doc = 4*g + b
        out[c * NDOC:(c + 1) * NDOC] = grid.T.reshape(-1)
    return out


if __name__ == "__main__":
    # quick CoreSim correctness check on a reduced doc count
    from concourse.bass_interp import CoreSim

    nd = int(os.environ.get("SIM_NDOC", "32"))
    rng = np.random.default_rng(0)

    def l2n(x):
        return x / np.sqrt((x * x).sum(-1, keepdims=True) + 1e-12)

    inputs = {
        "query_embs": l2n(rng.standard_normal((NQ, D))).astype(np.float32),
        "doc_embs": l2n(rng.standard_normal((nd, ND, D))).astype(np.float32),
        "importance_weights": rng.random(NQ).astype(np.float32),
        "query_mask": np.ones(NQ, bool),
        "doc_mask": np.ones((nd, ND), bool),
        "asc_w1": (rng.standard_normal((4, 32)) * 0.1).astype(np.float32),
        "asc_b1": np.zeros(32, np.float32),
        "asc_w2": (rng.standard_normal(32) * 0.1).astype(np.float32),
        "asc_b2": np.float32(0.0),
        "asc_blend": np.float32(0.5),
        "mgs_logits": (rng.standard_normal(3) * 0.1).astype(np.float32),
        "tir_w1": (rng.standard_normal((NQ, 64)) * 0.1).astype(np.float32),
        "tir_b1": np.zeros(64, np.float32),
        "tir_w2": (rng.standard_normal(64) * 0.1).astype(np.float32),
        "tir_b2": np.float32(0.0),
    }
    globals()["N"] = nd  # shrink problem for sim
    expected = _numpy_fallback(inputs)

    nc = _build_program(nd)
    in_maps = _host_prep(inputs, nd, 1)
    sim = CoreSim(nc)
    for k, v in in_maps[0].items():
        sim.tensor(k)[:] = v
    sim.simulate()
    grid = np.array(sim.tensor("out"))
    got = grid.T.reshape(-1)
    err = np.abs(got - expected)
    rel = err.max() / np.abs(expected).max()
    print("expected[:8]:", expected[:8])
    print("got[:8]     :", got[:8])
    print("max abs err:", err.max(), " rel:", rel)
    print("sim time (ns):", sim.time)


# revision 18
# speedup vs baseline: 1.0229x; 1.0229x over previous
"""Trainium2 Bass kernel for late-interaction retrieval scoring (FLUKE+).

Math per doc n (see reference):
  sims[q,t] = q_emb[q] . doc[n,t]                       (late interaction)
  pts[q]    = soft-top3 aggregation of sims[q,:]        (softmax(top3/T).top3)
  base      = sum_q wq[q] pts[q]
  ASC: pmax = max_t sims; stats (mean/max/std/frac) -> MLP -> calib;
       asc = (sum_q wq pmax) * (1+tanh(...)); total = blend*base+(1-blend)*asc
  MGS: for k=1..3, k-gram mean-pooled renormalized doc embs, MaxSim:
       sims_k[q,t] = (sum_i sims[q,t+i]) / ||sum_i doc[t+i]||
       total += gw[k] * sum_q wq max_t sims_k
  TIR: total += relu(pts*qm @ w1 + b1) @ w2 + b2

Device mapping (per core, NDOC docs), v2:
  - doc embeddings host-transposed to [D=128, t] (bf16); contraction dim D
    on SBUF partitions. 4 docs per PSUM sims tile via col-tiled matmuls
    (M=32 each, tile_position=(0,32b)).
  - k-gram numerators num2/num3 = shifted adds of sims, done on GPSIMD
    (alignment-immune) from an ACT-copied bf16 sims tile.
  - pairwise grams pp1/pp2 on DVE (2x mode); D-reduction via
    one-hot-column ones matmuls (one 358-col MM per doc);
    3-gram norm combined on GPSIMD; rsqrt = exp(-0.5*ln(2x+{2,3})) on ACT
    at doc resolution.
  - inv broadcast to all 32 q-rows: ONE [128x128] selector matmul per
    group (357 cols) instead of per-doc broadcasts.
  - scr_k = num_k * inv_k (one DVE mult per 2 groups, invb read straight
    from PSUM across 2 banks) + one segmented reduce_max into redc[...,2].
  - top-3 via DVE max8 on the bf16 sims copy.
  - finishing phase (per core, once): softmax over top3, weighted stats via
    col+row-tiled matmuls, ASC/TIR MLPs, combine, all on [*,128] tiles.
"""

import os
import numpy as np

# ---- problem constants (hardcoded; kernel.py must be self-contained)
N, NQ, ND, D = 4096, 32, 180, 128
TOPK, TEMP, MAXK = 3, 0.1, 3
NCORES = 8
NDOC = N // NCORES            # 512 docs per core
GROUP = 4                     # docs packed per col-tiled psum tile
SUPER = 32                    # docs per supergroup
NW2, NW3 = ND - 1, ND - 2     # 179, 178
NW = NW2 + NW3                # 357
BF = np.float16

_CACHE = {}


def _build_program(ndoc):
    """Build the SPMD Bass program for one core processing `ndoc` docs."""
    import concourse.bass as bass
    import concourse.tile as tile
    from concourse import mybir
    from contextlib import ExitStack

    f32, bf16 = mybir.dt.float32, mybir.dt.float16
    AF = mybir.ActivationFunctionType
    OP = mybir.AluOpType

    NSG = ndoc // SUPER       # supergroups
    NG = ndoc // GROUP        # total groups (= finishing tile width)
    GPS = SUPER // GROUP      # groups per supergroup = 8
    use_gps = os.environ.get("NO_GPSIMD", "0") != "1"
    use_ttr = os.environ.get("NO_TTR", "0") != "1"

    nc = bass.Bass()

    # ---------------- DRAM I/O ----------------
    docT = nc.dram_tensor("docT", [128, ndoc * ND], bf16, kind="ExternalInput")
    qT_d = nc.dram_tensor("qT", [128, NQ], bf16, kind="ExternalInput")
    selg_d = nc.dram_tensor("selg", [128, 32 * GPS], bf16, kind="ExternalInput")
    sful_d = nc.dram_tensor("sful", [128, 128 * GPS], bf16, kind="ExternalInput")
    wq32_d = nc.dram_tensor("wq32", [128, 32], f32, kind="ExternalInput")
    qmn32_d = nc.dram_tensor("qmn32", [128, 32], f32, kind="ExternalInput")
    tirw1_d = nc.dram_tensor("tirw1", [128, 64], f32, kind="ExternalInput")
    tirw2_d = nc.dram_tensor("tirw2", [128, 32], f32, kind="ExternalInput")
    tirb1_d = nc.dram_tensor("tirb1", [128, 1], f32, kind="ExternalInput")
    ascw1_d = nc.dram_tensor("ascw1", [128, 96], f32, kind="ExternalInput")
    ascw2_d = nc.dram_tensor("ascw2", [128, 32], f32, kind="ExternalInput")
    ascb1_d = nc.dram_tensor("ascb1", [128, 1], f32, kind="ExternalInput")
    ascb2x2_d = nc.dram_tensor("ascb2x2", [128, 1], f32, kind="ExternalInput")
    tirb2_d = nc.dram_tensor("tirb2", [128, 1], f32, kind="ExternalInput")
    qm4_d = nc.dram_tensor("qm4", [128, 1], f32, kind="ExternalInput")
    params_d = nc.dram_tensor("params", [1, 8], f32, kind="ExternalInput")
    out_d = nc.dram_tensor("out", [GROUP, NG], f32, kind="ExternalOutput")

    docT_v = docT.rearrange("p (d t) -> p d t", t=ND)

    with ExitStack() as ctx:
        tc = ctx.enter_context(tile.TileContext(nc))
        const = ctx.enter_context(tc.tile_pool(name="const", bufs=1))
        dpool = ctx.enter_context(tc.tile_pool(name="dpool", bufs=3))
        ppool = ctx.enter_context(tc.tile_pool(name="ppool", bufs=2))
        gpool = ctx.enter_context(tc.tile_pool(name="gpool", bufs=3))
        work = ctx.enter_context(tc.tile_pool(name="work", bufs=6))
        coll = ctx.enter_context(tc.tile_pool(name="coll", bufs=1))
        fin = ctx.enter_context(tc.tile_pool(name="fin", bufs=1))
        mainps = ctx.enter_context(ExitStack())
        ps = mainps.enter_context(tc.tile_pool(name="ps", bufs=3, space="PSUM"))
        psb = mainps.enter_context(tc.tile_pool(name="psb", bufs=2, space="PSUM"))
        psg = mainps.enter_context(tc.tile_pool(name="psg", bufs=1, space="PSUM"))

        # ---------------- constants ----------------
        qT = const.tile([128, NQ], bf16)
        nc.sync.dma_start(out=qT, in_=qT_d[:])
        selg = const.tile([128, 32 * GPS], bf16)
        nc.sync.dma_start(out=selg, in_=selg_d[:])
        sful = const.tile([128, 128 * GPS], bf16)
        nc.sync.dma_start(out=sful, in_=sful_d[:])
        wq32 = const.tile([128, 32], f32)
        nc.sync.dma_start(out=wq32, in_=wq32_d[:])
        qmn32 = const.tile([128, 32], f32)
        nc.sync.dma_start(out=qmn32, in_=qmn32_d[:])
        tirw1 = const.tile([128, 64], f32)
        nc.sync.dma_start(out=tirw1, in_=tirw1_d[:])
        tirw2 = const.tile([128, 32], f32)
        nc.sync.dma_start(out=tirw2, in_=tirw2_d[:])
        tirb1 = const.tile([128, 1], f32)
        nc.sync.dma_start(out=tirb1, in_=tirb1_d[:])
        ascw1 = const.tile([128, 96], f32)
        nc.sync.dma_start(out=ascw1, in_=ascw1_d[:])
        ascw2 = const.tile([128, 32], f32)
        nc.sync.dma_start(out=ascw2, in_=ascw2_d[:])
        ascb1 = const.tile([128, 1], f32)
        nc.sync.dma_start(out=ascb1, in_=ascb1_d[:])
        ascb2x2 = const.tile([128, 1], f32)
        nc.sync.dma_start(out=ascb2x2, in_=ascb2x2_d[:])
        tirb2 = const.tile([128, 1], f32)
        nc.sync.dma_start(out=tirb2, in_=tirb2_d[:])
        qm4 = const.tile([128, 1], f32)
        nc.sync.dma_start(out=qm4, in_=qm4_d[:])
        pvec = const.tile([1, 8], f32)
        nc.sync.dma_start(out=pvec, in_=params_d[:])

        b_zero = const.tile([128, 1], f32)
        nc.vector.memset(b_zero, 0.0)
        b_two = const.tile([128, 1], f32)
        nc.vector.memset(b_two, 2.0)
        b_three = const.tile([128, 1], f32)
        nc.vector.memset(b_three, 3.0)
        b_eps = const.tile([128, 1], f32)
        nc.vector.memset(b_eps, 1e-6)
        ones_row = const.tile([1, 128], f32)
        nc.vector.memset(ones_row, 1.0)

        # ---------------- collectors ----------------
        top8c = coll.tile([128, NG, 8], f32)
        redc = coll.tile([128, NG, 2], f32)

        # ---------------- main loop ----------------
        for sg in range(NSG):
            d0 = sg * SUPER
            dt_ = dpool.tile([128, SUPER, ND], bf16, tag="dt")
            nc.gpsimd.dma_start(out=dt_, in_=docT_v[:, d0:d0 + SUPER, :])

            # pairwise grams; pp layout [128, d, 360]: pp1@[0:179] pp2@[180:358]
            pp = ppool.tile([128, SUPER, 360], bf16, tag="pp")
            nc.vector.tensor_mul(pp[:, :, 0:NW2],
                                 dt_[:, :, 0:NW2], dt_[:, :, 1:ND])
            nc.vector.tensor_mul(pp[:, :, 180:180 + NW3],
                                 dt_[:, :, 0:NW3], dt_[:, :, 2:ND])
            nc.vector.memset(pp[:, :, NW2:180], 0.0)

            # D-reduction: gsum[32b+j, 0:179]=g1, [180:358]=g2 for doc 4j+b
            gsum_t = psg.tile([128, 512], f32, tag="gsum")
            gsum = gsum_t[:, 0:358]
            for dd in range(SUPER):
                j, b = dd // GROUP, dd % GROUP
                sel = selg[:, 32 * j:32 * (j + 1)]
                nc.tensor.matmul(gsum[32 * b:32 * (b + 1), :], sel,
                                 pp[:, dd, 0:358], start=True, stop=True,
                                 tile_position=(0, 32 * b),
                                 skip_group_check=True)

            # norms -> inv = rsqrt at doc resolution
            # inv layout (184-aligned regions): R2@[0:184] R3@[184:368]
            g_sb = gpool.tile([128, 358], bf16, tag="gsb")
            nc.scalar.copy(g_sb, gsum)
            t3 = gpool.tile([128, NW3], bf16, tag="t3")
            nc.vector.tensor_add(t3, g_sb[:, 0:NW3], g_sb[:, 1:NW2])
            n3in = gpool.tile([128, NW3], bf16, tag="n3in")
            nc.vector.tensor_add(n3in, t3, g_sb[:, 180:358])
            lnn = gpool.tile([128, 388], f32, tag="lnn")
            nc.scalar.activation(out=lnn[:, 0:NW2], in_=g_sb[:, 0:NW2],
                                 func=AF.Ln, bias=b_two, scale=2.0)
            nc.scalar.activation(out=lnn[:, 204:204 + NW3], in_=n3in,
                                 func=AF.Ln, bias=b_three, scale=2.0)
            inv = gpool.tile([128, 408], bf16, tag="inv")
            nc.vector.memset(inv[:, NW2:204], 0.0)
            nc.vector.memset(inv[:, 204 + NW3:408], 0.0)
            nc.scalar.activation(out=inv[:, 0:NW2], in_=lnn[:, 0:NW2],
                                 func=AF.Exp, bias=b_zero, scale=-0.5)
            nc.scalar.activation(out=inv[:, 204:204 + NW3],
                                 in_=lnn[:, 204:204 + NW3],
                                 func=AF.Exp, bias=b_zero, scale=-0.5)

            for j in range(GPS):
                gg = GPS * sg + j
                p2 = j % 2
                p4 = j % 4
                if p4 == 0:
                    # 4-group shared sims/num tile (fp16); only col 562 (the
                    # one pad lane the 179-wide segments read) needs a value
                    sev = work.tile([128, 4, 592], bf16, tag="sev")
                    nc.vector.memset(sev[:, :, 562:563], -60000.0)
                    invb_sb = work.tile([128, 4, 408], bf16, tag="invsb")
                if p2 == 0:
                    # invb spans 2 PSUM banks per 2 groups
                    invb2g_t = psb.tile([128, 1024], f32, tag="invb")
                # sims@[0:180] and num2@[180:359] accumulate in one bank
                ps_sims_t = ps.tile([128, 512], f32, tag="sims")
                ps_sims = ps_sims_t[:, 0:ND]
                ps_num2 = ps_sims_t[:, ND:ND + NW2]
                for b in range(GROUP):
                    d = GROUP * j + b
                    rows = slice(32 * b, 32 * (b + 1))
                    tp = (0, 32 * b)
                    nc.tensor.matmul(ps_sims[rows, :], qT,
                                     dt_[:, d, :], start=True, stop=True,
                                     tile_position=tp, skip_group_check=True)
                    nc.tensor.matmul(ps_num2[rows, :], qT,
                                     dt_[:, d, 0:NW2], start=True, stop=False,
                                     tile_position=tp, skip_group_check=True)
                    nc.tensor.matmul(ps_num2[rows, :], qT,
                                     dt_[:, d, 1:ND], start=False, stop=True,
                                     tile_position=tp, skip_group_check=True)
                # inv rows {32b+j} broadcast to all 32 q-rows, one matmul
                nc.tensor.matmul(invb2g_t[:, 512 * p2:512 * p2 + 408],
                                 sful[:, 128 * j:128 * (j + 1)],
                                 inv, start=True, stop=True,
                                 tile_position=(0, 0), skip_group_check=True)
                if p2 == 1:
                    # both halves written: move invb to SBUF fp16 (ACT)
                    nc.scalar.copy(
                        invb_sb[:, 2 * (p4 // 2):2 * (p4 // 2) + 2, :],
                        invb2g_t.rearrange("p (g c) -> p g c",
                                           c=512)[:, :, 0:408])

                # sims+num2 -> SBUF fp16 in one ACT copy
                nc.scalar.copy(sev[:, p4, 0:359], ps_sims_t[:, 0:359])
                nc.vector.max(top8c[:, gg, :], sev[:, p4, 0:ND])

                if p4 == 3:
                    # num3 = num2 + s[t+2], one aligned 2x DVE add for 4 grps
                    nc.vector.tensor_add(sev[:, :, 384:384 + NW3],
                                         sev[:, :, 180:180 + NW3],
                                         sev[:, :, 2:ND])
                    # scr = num*inv (all-SBUF fp16) + segmented max, 4 grp/pass
                    scr4g = work.tile([128, 4, 368], bf16, tag="scr")
                    num_v = sev[:, :, 180:588].rearrange(
                        "p g (r c) -> p g r c", c=204)[:, :, :, 0:179]
                    invs_v = invb_sb.rearrange(
                        "p g (r c) -> p g r c", c=204)[:, :, :, 0:179]
                    scr_v = scr4g.rearrange(
                        "p g (r c) -> p g r c", c=184)[:, :, :, 0:179]
                    nc.vector.tensor_mul(scr_v, num_v, invs_v)
                    nc.vector.reduce_max(
                        redc[:, gg - 3:gg + 1, :], scr_v,
                        axis=mybir.AxisListType.X)

        # ---------------- finishing phase ----------------
        mainps.close()
        ps = ctx.enter_context(tc.tile_pool(name="psf", bufs=1, space="PSUM"))

        topv = top8c[:, :, 0:TOPK]              # [128, NG, 3]
        pmax = top8c[:, :, 0:1]                 # [128, NG, 1]

        # collector-ready stats first, so the PE runs under the pts chain
        pmax2 = fin.tile([128, NG], f32)
        nc.vector.tensor_mul(pmax2, pmax[:, :, 0], pmax[:, :, 0])
        ps_stat_t = ps.tile([128, 512], f32, tag="stat")
        ps_stat = ps_stat_t[:, 0:3 * NG]        # asc_base | mean | msq
        ps_stat2_t = ps.tile([128, 512], f32, tag="stat2")
        ps_stat2 = ps_stat2_t[:, 0:3 * NG]      # red2w | red3w | base
        ps_mlp_t = ps.tile([128, 512], f32, tag="mlp")
        ps_mlp = ps_mlp_t[:, 0:NG]              # H
        ps_base = ps_stat2[:, 2 * NG:3 * NG]
        pmax_2d = top8c[:, :, 0]                # [128, NG] strided view
        for b in range(GROUP):
            rows = slice(32 * b, 32 * (b + 1))
            tp = (32 * b, 32 * b)
            nc.tensor.matmul(ps_stat[rows, 0:NG], wq32[rows, :],
                             pmax_2d[rows, :], start=True, stop=True,
                             tile_position=tp, skip_group_check=True)
            nc.tensor.matmul(ps_stat[rows, NG:2 * NG], qmn32[rows, :],
                             pmax_2d[rows, :], start=True, stop=True,
                             tile_position=tp, skip_group_check=True)
            nc.tensor.matmul(ps_stat[rows, 2 * NG:3 * NG], qmn32[rows, :],
                             pmax2[rows, :], start=True, stop=True,
                             tile_position=tp, skip_group_check=True)
            nc.tensor.matmul(ps_stat2[rows, 0:NG], wq32[rows, :],
                             redc[rows, :, 0], start=True, stop=True,
                             tile_position=tp, skip_group_check=True)
            nc.tensor.matmul(ps_stat2[rows, NG:2 * NG], wq32[rows, :],
                             redc[rows, :, 1], start=True, stop=True,
                             tile_position=tp, skip_group_check=True)

        # pts = softmax(topv/T).topv
        exps = fin.tile([128, NG, TOPK], f32)
        nc.scalar.activation(out=exps, in_=topv, func=AF.Exp,
                             bias=b_zero, scale=1.0 / TEMP)
        den = fin.tile([128, NG], f32)
        nc.vector.reduce_sum(den, exps, axis=mybir.AxisListType.X)
        wnum = fin.tile([128, NG, TOPK], f32)
        nc.vector.tensor_mul(wnum, exps, topv)
        pnum = fin.tile([128, NG], f32)
        nc.vector.reduce_sum(pnum, wnum, axis=mybir.AxisListType.X)
        rden = fin.tile([128, NG], f32)
        nc.vector.reciprocal(rden, den)
        pts = fin.tile([128, NG], f32)
        nc.vector.tensor_mul(pts, pnum, rden)
        # padded = pts * qm (qm broadcast per partition block)
        pts_t = fin.tile([128, NG], f32)
        nc.vector.tensor_scalar(out=pts_t, in0=pts, scalar1=qm4, scalar2=None,
                                op0=OP.mult)
        for b in range(GROUP):
            rows = slice(32 * b, 32 * (b + 1))
            tp = (32 * b, 32 * b)
            nc.tensor.matmul(ps_base[rows, :], wq32[rows, :],
                             pts[rows, :], start=True, stop=True,
                             tile_position=tp, skip_group_check=True)

        # single ordered whole-bank reads (PSUM banks must not be read while
        # PE still writes other columns of the same bank)
        stat_sb = fin.tile([128, 3 * NG], f32)
        nc.scalar.copy(stat_sb, ps_stat)
        stat2_sb = fin.tile([128, 3 * NG], f32)
        nc.scalar.copy(stat2_sb, ps_stat2)
        # mx: max over q (transpose + free-dim reduce + transpose back)
        ident = const.tile([128, 128], f32)
        nc.sync.dma_start(out=ident, in_=nc.dram_tensor(
            "ident", [128, 128], f32, kind="ExternalInput")[:])
        pmax_sb = fin.tile([128, NG], f32)
        nc.vector.tensor_copy(pmax_sb, pmax_2d)
        ps_pT_t = ps.tile([128, 512], f32, tag="ptrans")
        ps_pT = ps_pT_t[:, 0:128]
        nc.tensor.transpose(ps_pT[0:NG, :], pmax_sb, ident)
        mxT = fin.tile([128, GROUP], f32)
        nc.vector.reduce_max(mxT[0:NG, :], ps_pT[0:NG, :].rearrange(
            "g (b q) -> g b q", q=32), axis=mybir.AxisListType.X)
        mxpad = fin.tile([128, 128], f32)
        nc.vector.memset(mxpad, 0.0)
        nc.vector.tensor_copy(
            mxpad[0:NG, :].rearrange("g (b o) -> g b o", o=32)[:, :, 0:1],
            mxT[0:NG, :].rearrange("g (b o) -> g b o", o=1))
        ps_mxb_t = ps.tile([128, 512], f32, tag="ptrans")
        ps_mxb = ps_mxb_t[:, 0:128]
        nc.tensor.transpose(ps_mxb[:, 0:NG], mxpad[0:NG, :], ident[0:NG, 0:NG])

        # ---- per-strip finishing: every accessed row is at base 32b ----
        F = fin.tile([128, 3 * NG], f32)    # cols: mean | mx | std, rows {32b}
        scrA = fin.tile([128, NG], f32)
        scrB = fin.tile([128, NG], f32)
        for b in range(GROUP):
            row = slice(32 * b, 32 * b + 1)
            mean = stat_sb[row, NG:2 * NG]
            msq = stat_sb[row, 2 * NG:3 * NG]
            nc.vector.tensor_copy(F[row, 0:NG], mean)
            nc.scalar.copy(F[row, NG:2 * NG], ps_mxb[row, 0:NG])
            # var = msq - mean^2 -> std = exp(0.5*ln(var + 1e-6))
            nc.vector.tensor_mul(scrA[row, :], F[row, 0:NG], F[row, 0:NG])
            nc.vector.tensor_sub(scrB[row, :], msq, scrA[row, :])
            nc.scalar.activation(out=scrA[row, :], in_=scrB[row, :],
                                 func=AF.Ln, bias=b_eps[row, :], scale=1.0)
            nc.scalar.activation(out=F[row, 2 * NG:3 * NG], in_=scrA[row, :],
                                 func=AF.Exp, bias=b_zero[row, :], scale=0.5)

        # ASC MLP via K=1 accumulation over the 3 features
        for b in range(GROUP):
            row = slice(32 * b, 32 * b + 1)
            for s in range(3):
                nc.tensor.matmul(ps_mlp[32 * b:32 * (b + 1), :],
                                 ascw1[row, 32 * s:32 * (s + 1)],
                                 F[row, s * NG:(s + 1) * NG],
                                 start=(s == 0), stop=(s == 2),
                                 tile_position=(32 * b, 32 * b),
                                 skip_group_check=True)
        Hs = fin.tile([128, NG], f32)
        nc.scalar.activation(out=Hs, in_=ps_mlp, func=AF.Relu, bias=ascb1, scale=1.0)
        ps_calsc_t = ps.tile([128, 512], f32, tag="cal")
        ps_cal = ps_calsc_t[:, 0:NG]
        ps_sc = ps_calsc_t[:, NG:NG + 8]
        for b in range(GROUP):
            nc.tensor.matmul(ps_cal[32 * b:32 * (b + 1), :],
                             ascw2[32 * b:32 * (b + 1), :],
                             Hs[32 * b:32 * (b + 1), :], start=True, stop=True,
                             tile_position=(32 * b, 32 * b), skip_group_check=True)
        # gw = softmax(mgs_logits); blend = sigmoid(asc_blend)
        ge = fin.tile([1, 3], f32)
        gden = fin.tile([1, 1], f32)
        nc.scalar.activation(out=ge, in_=pvec[0:1, 0:3], func=AF.Exp,
                             bias=b_zero[0:1, :], scale=1.0)
        nc.vector.reduce_sum(gden, ge, axis=mybir.AxisListType.X)
        grden = fin.tile([1, 1], f32)
        nc.vector.reciprocal(grden, gden)
        svec = fin.tile([1, 8], f32)
        nc.vector.memset(svec, 0.0)
        nc.vector.tensor_scalar(out=svec[0:1, 0:3], in0=ge, scalar1=grden,
                                scalar2=None, op0=OP.mult)
        be = fin.tile([1, 1], f32)
        nc.scalar.activation(out=be, in_=pvec[0:1, 3:4], func=AF.Exp,
                             bias=b_zero[0:1, :], scale=-1.0)
        nc.vector.tensor_scalar(out=be, in0=be, scalar1=1.0, scalar2=None, op0=OP.add)
        nc.vector.reciprocal(svec[0:1, 3:4], be)                       # blend
        nc.vector.tensor_scalar(out=svec[0:1, 4:5], in0=svec[0:1, 3:4],
                                scalar1=-1.0, scalar2=1.0, op0=OP.mult, op1=OP.add)
        nc.tensor.matmul(ps_sc, ones_row, svec, start=True, stop=True,
                         tile_position=(0, 0), skip_group_check=True)
        calsc_sb = fin.tile([128, NG + 8], f32)
        nc.scalar.copy(calsc_sb, ps_calsc_t[:, 0:NG + 8])
        sc = calsc_sb[:, NG:NG + 8]
        cal_sb = calsc_sb[:, 0:NG]

        # TIR MLP (row-tiled K=32); A/B in separate banks (PE-W vs ACT-R hazard)
        ps_tirA_t = ps.tile([128, 512], f32, tag="tirA")
        ps_tirA = ps_tirA_t[:, 0:NG]
        ps_tirB_t = ps.tile([128, 512], f32, tag="tirB")
        ps_tirB = ps_tirB_t[:, 0:NG]
        tir_dst = [(ps_tirA, 0), (ps_tirA, 64), (ps_tirB, 0), (ps_tirB, 64)]
        for b in range(GROUP):
            dst, o = tir_dst[b]
            nc.tensor.matmul(dst[o:o + 64, :], tirw1[32 * b:32 * (b + 1), :],
                             pts_t[32 * b:32 * (b + 1), :], start=True, stop=True,
                             tile_position=(32 * b, o), skip_group_check=True)
        HsA = fin.tile([128, NG], f32)
        HsB = fin.tile([128, NG], f32)
        nc.scalar.activation(out=HsA, in_=ps_tirA, func=AF.Relu, bias=tirb1, scale=1.0)
        nc.scalar.activation(out=HsB, in_=ps_tirB, func=AF.Relu, bias=tirb1, scale=1.0)
        ps_tv_t = ps.tile([128, 512], f32, tag="tv")
        ps_tv = ps_tv_t[:, 0:NG]
        tir_src = [(HsA, 0), (HsA, 64), (HsB, 0), (HsB, 64)]
        for b in range(GROUP):
            src_t, o = tir_src[b]
            nc.tensor.matmul(ps_tv[32 * b:32 * (b + 1), :], tirw2[o:o + 64, :],
                             src_t[o:o + 64, :], start=True, stop=True,
                             tile_position=(o, 32 * b), skip_group_check=True)

        tv_sb = fin.tile([128, NG], f32)
        nc.scalar.copy(tv_sb, ps_tv)
        # combine per strip on row {32b}:
        # total = blend*base + (1-blend)*asc_base*(1+calib)
        #         + gw0*asc_base + gw1*red2w + gw2*red3w + tirv + tir_b2
        tot = fin.tile([128, NG], f32)
        for b in range(GROUP):
            row = slice(32 * b, 32 * b + 1)
            asc_base = stat_sb[row, 0:NG]
            base = stat2_sb[row, 2 * NG:3 * NG]
            # calib1 = 1 + tanh(x + b2) = 2 - 2/(exp(2x+2b2)+1)
            nc.scalar.activation(out=scrA[row, :], in_=cal_sb[row, :],
                                 func=AF.Exp, bias=ascb2x2[row, :], scale=2.0)
            nc.vector.tensor_scalar(out=scrA[row, :], in0=scrA[row, :],
                                    scalar1=1.0, scalar2=None, op0=OP.add)
            nc.vector.reciprocal(scrB[row, :], scrA[row, :])
            nc.vector.tensor_scalar(out=scrB[row, :], in0=scrB[row, :],
                                    scalar1=-2.0, scalar2=2.0, op0=OP.mult, op1=OP.add)
            nc.vector.tensor_mul(scrB[row, :], scrB[row, :], asc_base)
            nc.vector.tensor_scalar(out=scrB[row, :], in0=scrB[row, :],
                                    scalar1=sc[row, 4:5], scalar2=None, op0=OP.mult)
            nc.vector.tensor_scalar(out=scrA[row, :], in0=base,
                                    scalar1=sc[row, 3:4], scalar2=None, op0=OP.mult)
            nc.vector.tensor_add(tot[row, :], scrA[row, :], scrB[row, :])
            nc.vector.tensor_scalar(out=scrA[row, :], in0=asc_base,
                                    scalar1=sc[row, 0:1], scalar2=None, op0=OP.mult)
            nc.vector.tensor_add(tot[row, :], tot[row, :], scrA[row, :])
            nc.vector.tensor_scalar(out=scrA[row, :], in0=stat2_sb[row, 0:NG],
                                    scalar1=sc[row, 1:2], scalar2=None, op0=OP.mult)
            nc.vector.tensor_add(tot[row, :], tot[row, :], scrA[row, :])
            nc.vector.tensor_scalar(out=scrA[row, :], in0=stat2_sb[row, NG:2 * NG],
                                    scalar1=sc[row, 2:3], scalar2=None, op0=OP.mult)
            nc.vector.tensor_add(tot[row, :], tot[row, :], scrA[row, :])
            nc.vector.tensor_add(tot[row, :], tot[row, :], tv_sb[row, :])
            nc.vector.tensor_scalar(out=tot[row, :], in0=tot[row, :],
                                    scalar1=tirb2[row, :], scalar2=None, op0=OP.add)
            nc.sync.dma_start(out=out_d[b:b + 1, :], in_=tot[row, :])

    return nc


def _legalize_single_wait(nc):
    """Walrus (this compile path) accepts at most one sync wait per
    instruction; offload extra waits onto preceding EventSemaphore
    instructions on the same engine queue."""
    from concourse import mybir
    ctr = [0]
    for bb in nc.main_func.blocks:
        il = bb.instructions
        out = []
        for inst in il:
            si = inst.sync_info
            if si is not None and len(si.on_wait) > 1:
                waits = list(si.on_wait)
                eng = nc.engines[inst.engine]
                for w in waits[:-1]:
                    ev = eng._isa(
                        nc.isa.Opcode.NEURON_ISA_TPB_OPCODE_NOP, {})
                    ev.sync_info = mybir.SyncInfo(on_wait=[w], on_update=[])
                    ctr[0] += 1
                    try:
                        nc.register_instruction(ev)
                    except Exception:
                        pass
                    out.append(ev)
                inst.sync_info = mybir.SyncInfo(on_wait=[waits[-1]],
                                                on_update=list(si.on_update))
            out.append(inst)
        bb.instructions = out
    return nc


def _host_prep(inputs, ndoc_per_core, ncores):
    """Shard + lay out inputs for the SPMD program. Returns list of in_maps."""
    q = np.asarray(inputs["query_embs"], np.float32)          # [NQ, D]
    docs = np.asarray(inputs["doc_embs"], np.float32)         # [N, ND, D]
    w = np.asarray(inputs["importance_weights"], np.float32)  # [NQ]
    qm = np.asarray(inputs["query_mask"]).astype(np.float32)  # [NQ]
    asc_w1 = np.asarray(inputs["asc_w1"], np.float32)
    asc_b1 = np.asarray(inputs["asc_b1"], np.float32)
    asc_w2 = np.asarray(inputs["asc_w2"], np.float32)
    asc_b2 = np.float32(inputs["asc_b2"])
    asc_blend = np.float32(inputs["asc_blend"])
    mgs_logits = np.asarray(inputs["mgs_logits"], np.float32)
    tir_w1 = np.asarray(inputs["tir_w1"], np.float32)
    tir_b1 = np.asarray(inputs["tir_b1"], np.float32)
    tir_w2 = np.asarray(inputs["tir_w2"], np.float32)
    tir_b2 = np.float32(inputs["tir_b2"])

    wq = (w * qm).astype(np.float32)
    nvalid = float(qm.sum())
    frac = nvalid / NQ

    # frac folded into ASC layer-1 bias; feats order = [mean, mx, std]
    b1p = asc_b1 + frac * asc_w1[3, :]
    w1p = asc_w1[:3, :]                                       # [3, 32]

    GPS = SUPER // GROUP
    qT = np.ascontiguousarray(q.T).astype(BF)                 # [128, 32]
    selg = np.zeros((128, 32 * GPS), BF)
    for j in range(GPS):
        selg[:, 32 * j + j] = 1
    # sful: S_j[32b+j, 128j + 32b+q] = 1 (bcast inv row 32b+j -> rows 32b+q)
    sful = np.zeros((128, 128 * GPS), BF)
    for j in range(GPS):
        for b in range(GROUP):
            sful[32 * b + j, 128 * j + 32 * b:128 * j + 32 * (b + 1)] = 1
    wq32 = np.zeros((128, 32), np.float32)
    wq32[:, 0] = np.tile(wq, 4)
    qmn32 = np.zeros((128, 32), np.float32)
    qmn32[:, 0] = np.tile(qm / max(nvalid, 1e-9), 4)
    tirw1 = np.tile(tir_w1, (4, 1)).astype(np.float32)        # [128, 64]
    tirw2 = np.zeros((128, 32), np.float32)
    tirw2[:, 0] = np.tile(tir_w2, 2)
    tirb1 = np.tile(tir_b1, 2).reshape(128, 1).astype(np.float32)
    ascw1 = np.zeros((128, 96), np.float32)
    for b in range(4):
        for s in range(3):
            ascw1[32 * b, 32 * s:32 * (s + 1)] = w1p[s, :]
    ascw2 = np.zeros((128, 32), np.float32)
    ascw2[:, 0] = np.tile(asc_w2, 4)
    ascb1 = np.tile(b1p, 4).reshape(128, 1).astype(np.float32)
    ascb2x2 = np.full((128, 1), 2.0 * asc_b2, np.float32)
    tirb2 = np.full((128, 1), tir_b2, np.float32)
    qm4 = np.tile(qm, 4).reshape(128, 1).astype(np.float32)
    params = np.zeros((1, 8), np.float32)
    params[0, 0:3] = mgs_logits
    params[0, 3] = asc_blend
    ident = np.eye(128, dtype=np.float32)

    shared = dict(qT=qT, selg=selg, sful=sful, wq32=wq32, qmn32=qmn32,
                  tirw1=tirw1, tirw2=tirw2, tirb1=tirb1, ascw1=ascw1,
                  ascw2=ascw2, ascb1=ascb1, ascb2x2=ascb2x2, tirb2=tirb2,
                  qm4=qm4, params=params, ident=ident)

    in_maps = []
    for c in range(ncores):
        sl = docs[c * ndoc_per_core:(c + 1) * ndoc_per_core]   # [ndoc, ND, D]
        dT = np.ascontiguousarray(sl.transpose(2, 0, 1)).astype(BF)
        m = dict(shared)
        m["docT"] = dT.reshape(128, ndoc_per_core * ND)
        in_maps.append(m)
    return in_maps


def _numpy_fallback(inputs):
    """Full-precision numpy implementation (only for non-all-ones masks)."""
    q = np.asarray(inputs["query_embs"], np.float64)
    docs = np.asarray(inputs["doc_embs"], np.float64)
    w = np.asarray(inputs["importance_weights"], np.float64)
    qm_b = np.asarray(inputs["query_mask"]).astype(bool)
    dm_b = np.asarray(inputs["doc_mask"]).astype(bool)
    NEG = -1e9
    qm = qm_b.astype(np.float64)
    wq = w * qm
    sims = np.einsum("qd,ntd->nqt", q, docs)
    sims = np.where(dm_b[:, None, :], sims, NEG)
    topv = -np.sort(-sims, axis=-1)[:, :, :TOPK]
    e = np.exp((topv - topv.max(-1, keepdims=True)) / TEMP)
    soft = e / e.sum(-1, keepdims=True)
    pts = (soft * topv).sum(-1)
    base = pts @ wq
    pmax = sims.max(-1)
    asc_base = pmax @ wq
    nvalid = qm.sum()
    mean = (pmax * qm).sum(-1) / nvalid
    mx = np.where(qm_b, pmax, NEG).max(-1)
    std = np.sqrt((((pmax - mean[:, None]) ** 2) * qm).sum(-1) / nvalid + 1e-6)
    frac = np.full_like(mean, nvalid / NQ)
    feats = np.stack([mean, mx, std, frac], -1)
    h = np.maximum(feats @ np.asarray(inputs["asc_w1"], np.float64)
                   + np.asarray(inputs["asc_b1"], np.float64), 0)
    calib = np.tanh(h @ np.asarray(inputs["asc_w2"], np.float64)
                    + float(inputs["asc_b2"]))
    asc_score = asc_base * (1.0 + calib)
    blend = 1 / (1 + np.exp(-float(inputs["asc_blend"])))
    total = blend * base + (1 - blend) * asc_score
    gl = np.asarray(inputs["mgs_logits"], np.float64)
    gw = np.exp(gl - gl.max()); gw /= gw.sum()
    dmf = dm_b.astype(np.float64)
    for k in range(1, MAXK + 1):
        if k == 1:
            dk, mk = docs, dm_b
        else:
            nw = ND - k + 1
            s = sum(docs[:, i:i + nw] for i in range(k)) / k
            dk = s / np.sqrt((s * s).sum(-1, keepdims=True) + 1e-12)
            mkf = dmf[:, 0:nw].copy()
            for i in range(1, k):
                mkf = mkf * dmf[:, i:i + nw]
            mk = mkf > 0.5
        sk = np.einsum("qd,nwd->nqw", q, dk)
        sk = np.where(mk[:, None, :], sk, NEG)
        total = total + gw[k - 1] * (sk.max(-1) @ wq)
    padded = pts * qm
    hres = np.maximum(padded @ np.asarray(inputs["tir_w1"], np.float64)
                      + np.asarray(inputs["tir_b1"], np.float64), 0)
    total = total + hres @ np.asarray(inputs["tir_w2"], np.float64) + float(inputs["tir_b2"])
    return total.astype(np.float32)


def kernel(**inputs):
    qm = np.asarray(inputs["query_mask"]).astype(bool)
    dm = np.asarray(inputs["doc_mask"]).astype(bool)
    if not (qm.all() and dm.all()):
        return _numpy_fallback(inputs)

    from concourse.bass_utils import run_bass_kernel_spmd

    key = ("prog", NDOC)
    if key not in _CACHE:
        from concourse import mybir
        nc = _legalize_single_wait(_build_program(NDOC))
        # populate .instr bytes for extended-inst ISA subclasses
        # (tensor_tensor_reduce); without this walrus fails with
        # "ISA wrong length".
        mybir.codegen_inst_isa_subclasses(nc)
        _CACHE[key] = nc
    nc = _CACHE[key]

    in_maps = _host_prep(inputs, NDOC, NCORES)
    res = run_bass_kernel_spmd(nc, in_maps, list(range(NCORES)))
    out = np.empty((N,), np.float32)
    for c in range(NCORES):
        grid = res.results[c]["out"]            # [4, NG] -> doc = 4*g + b
        out[c * NDOC:(c + 1) * NDOC] = grid.T.reshape(-1)
    return out


if __name__ == "__main__":
    # quick CoreSim correctness check on a reduced doc count
    from concourse.bass_interp import CoreSim

    nd = int(os.environ.get("SIM_NDOC", "32"))
    rng = np.random.default_rng(0)

    def l2n(x):
        return x / np.sqrt((x * x).sum(-1, keepdims=True) + 1e-12)

    inputs = {
        "query_embs": l2n(rng.standard_normal((NQ, D))).astype(np.float32),
        "doc_embs": l2n(rng.standard_normal((nd, ND, D))).astype(np.float32),
        "importance_weights": rng.random(NQ).astype(np.float32),
        "query_mask": np.ones(NQ, bool),
        "doc_mask": np.ones((nd, ND), bool),
        "asc_w1": (rng.standard_normal((4, 32)) * 0.1).astype(np.float32),
        "asc_b1": np.zeros(32, np.float32),
        "asc_w2": (rng.standard_normal(32) * 0.1).astype(np.float32),
        "asc_b2": np.float32(0.0),
        "asc_blend": np.float32(0.5),
        "mgs_logits": (rng.standard_normal(3) * 0.1).astype(np.float32),
        "tir_w1": (rng.standard_normal((NQ, 64)) * 0.1).astype(np.float32),
        "tir_b1": np.zeros(64, np.float32),
        "tir_w2": (rng.standard_normal(64) * 0.1).astype(np.float32),
        "tir_b2": np.float32(0.0),
    }
    globals()["N"] = nd  # shrink problem for sim
    expected = _numpy_fallback(inputs)

    nc = _build_program(nd)
    in_maps = _host_prep(inputs, nd, 1)
    sim = CoreSim(nc)
    for k, v in in_maps[0].items():
        sim.tensor(k)[:] = v
    sim.simulate()
    grid = np.array(sim.tensor("out"))
    got = grid.T.reshape(-1)
    err = np.abs(got - expected)
    rel = err.max() / np.abs(expected).max()
    print("expected[:8]:", expected[:8])
    print("got[:8]     :", got[:8])
    print("max abs err:", err.max(), " rel:", rel)
    print("sim time (ns):", sim.time)


# revision 21
# speedup vs baseline: 1.0414x; 1.0180x over previous
"""Trainium2 Bass kernel for late-interaction retrieval scoring (FLUKE+).

Math per doc n (see reference):
  sims[q,t] = q_emb[q] . doc[n,t]                       (late interaction)
  pts[q]    = soft-top3 aggregation of sims[q,:]        (softmax(top3/T).top3)
  base      = sum_q wq[q] pts[q]
  ASC: pmax = max_t sims; stats (mean/max/std/frac) -> MLP -> calib;
       asc = (sum_q wq pmax) * (1+tanh(...)); total = blend*base+(1-blend)*asc
  MGS: for k=1..3, k-gram mean-pooled renormalized doc embs, MaxSim:
       sims_k[q,t] = (sum_i sims[q,t+i]) / ||sum_i doc[t+i]||
       total += gw[k] * sum_q wq max_t sims_k
  TIR: total += relu(pts*qm @ w1 + b1) @ w2 + b2

Device mapping (per core, NDOC docs), v2:
  - doc embeddings host-transposed to [D=128, t] (bf16); contraction dim D
    on SBUF partitions. 4 docs per PSUM sims tile via col-tiled matmuls
    (M=32 each, tile_position=(0,32b)).
  - k-gram numerators num2/num3 = shifted adds of sims, done on GPSIMD
    (alignment-immune) from an ACT-copied bf16 sims tile.
  - pairwise grams pp1/pp2 on DVE (2x mode); D-reduction via
    one-hot-column ones matmuls (one 358-col MM per doc);
    3-gram norm combined on GPSIMD; rsqrt = exp(-0.5*ln(2x+{2,3})) on ACT
    at doc resolution.
  - inv broadcast to all 32 q-rows: ONE [128x128] selector matmul per
    group (357 cols) instead of per-doc broadcasts.
  - scr_k = num_k * inv_k (one DVE mult per 2 groups, invb read straight
    from PSUM across 2 banks) + one segmented reduce_max into redc[...,2].
  - top-3 via DVE max8 on the bf16 sims copy.
  - finishing phase (per core, once): softmax over top3, weighted stats via
    col+row-tiled matmuls, ASC/TIR MLPs, combine, all on [*,128] tiles.
"""

import os
import numpy as np

# ---- problem constants (hardcoded; kernel.py must be self-contained)
N, NQ, ND, D = 4096, 32, 180, 128
TOPK, TEMP, MAXK = 3, 0.1, 3
NCORES = 8
NDOC = N // NCORES            # 512 docs per core
GROUP = 4                     # docs packed per col-tiled psum tile
SUPER = 32                    # docs per supergroup
NW2, NW3 = ND - 1, ND - 2     # 179, 178
NW = NW2 + NW3                # 357
BF = np.float16

_CACHE = {}


def _build_program(ndoc):
    """Build the SPMD Bass program for one core processing `ndoc` docs."""
    import concourse.bass as bass
    import concourse.tile as tile
    from concourse import mybir
    from contextlib import ExitStack

    f32, bf16 = mybir.dt.float32, mybir.dt.float16
    AF = mybir.ActivationFunctionType
    OP = mybir.AluOpType

    NSG = ndoc // SUPER       # supergroups
    NG = ndoc // GROUP        # total groups (= finishing tile width)
    GPS = SUPER // GROUP      # groups per supergroup = 8
    use_gps = os.environ.get("NO_GPSIMD", "0") != "1"
    use_ttr = os.environ.get("NO_TTR", "0") != "1"

    nc = bass.Bass()

    # ---------------- DRAM I/O ----------------
    docT = nc.dram_tensor("docT", [128, ndoc * ND], bf16, kind="ExternalInput")
    qT_d = nc.dram_tensor("qT", [128, NQ], bf16, kind="ExternalInput")
    selg_d = nc.dram_tensor("selg", [128, 32 * GPS], bf16, kind="ExternalInput")
    sful_d = nc.dram_tensor("sful", [128, 128 * GPS], bf16, kind="ExternalInput")
    wq32_d = nc.dram_tensor("wq32", [128, 32], f32, kind="ExternalInput")
    qmn32_d = nc.dram_tensor("qmn32", [128, 32], f32, kind="ExternalInput")
    tirw1_d = nc.dram_tensor("tirw1", [128, 64], f32, kind="ExternalInput")
    tirw2_d = nc.dram_tensor("tirw2", [128, 32], f32, kind="ExternalInput")
    tirb1_d = nc.dram_tensor("tirb1", [128, 1], f32, kind="ExternalInput")
    ascw1_d = nc.dram_tensor("ascw1", [128, 96], f32, kind="ExternalInput")
    ascw2_d = nc.dram_tensor("ascw2", [128, 32], f32, kind="ExternalInput")
    ascb1_d = nc.dram_tensor("ascb1", [128, 1], f32, kind="ExternalInput")
    ascb2x2_d = nc.dram_tensor("ascb2x2", [128, 1], f32, kind="ExternalInput")
    tirb2_d = nc.dram_tensor("tirb2", [128, 1], f32, kind="ExternalInput")
    qm4_d = nc.dram_tensor("qm4", [128, 1], f32, kind="ExternalInput")
    params_d = nc.dram_tensor("params", [1, 8], f32, kind="ExternalInput")
    out_d = nc.dram_tensor("out", [GROUP, NG], f32, kind="ExternalOutput")

    docT_v = docT.rearrange("p (d t) -> p d t", t=ND)

    with ExitStack() as ctx:
        tc = ctx.enter_context(tile.TileContext(nc))
        const = ctx.enter_context(tc.tile_pool(name="const", bufs=1))
        dpool = ctx.enter_context(tc.tile_pool(name="dpool", bufs=3))
        ppool = ctx.enter_context(tc.tile_pool(name="ppool", bufs=2))
        gpool = ctx.enter_context(tc.tile_pool(name="gpool", bufs=3))
        work = ctx.enter_context(tc.tile_pool(name="work", bufs=6))
        coll = ctx.enter_context(tc.tile_pool(name="coll", bufs=1))
        fin = ctx.enter_context(tc.tile_pool(name="fin", bufs=1))
        mainps = ctx.enter_context(ExitStack())
        ps = mainps.enter_context(tc.tile_pool(name="ps", bufs=3, space="PSUM"))
        psb = mainps.enter_context(tc.tile_pool(name="psb", bufs=2, space="PSUM"))
        psg = mainps.enter_context(tc.tile_pool(name="psg", bufs=1, space="PSUM"))

        # ---------------- constants ----------------
        qT = const.tile([128, NQ], bf16)
        nc.sync.dma_start(out=qT, in_=qT_d[:])
        selg = const.tile([128, 32 * GPS], bf16)
        nc.sync.dma_start(out=selg, in_=selg_d[:])
        sful = const.tile([128, 128 * GPS], bf16)
        nc.sync.dma_start(out=sful, in_=sful_d[:])
        wq32 = const.tile([128, 32], f32)
        nc.sync.dma_start(out=wq32, in_=wq32_d[:])
        qmn32 = const.tile([128, 32], f32)
        nc.sync.dma_start(out=qmn32, in_=qmn32_d[:])
        tirw1 = const.tile([128, 64], f32)
        nc.sync.dma_start(out=tirw1, in_=tirw1_d[:])
        tirw2 = const.tile([128, 32], f32)
        nc.sync.dma_start(out=tirw2, in_=tirw2_d[:])
        tirb1 = const.tile([128, 1], f32)
        nc.sync.dma_start(out=tirb1, in_=tirb1_d[:])
        ascw1 = const.tile([128, 96], f32)
        nc.sync.dma_start(out=ascw1, in_=ascw1_d[:])
        ascw2 = const.tile([128, 32], f32)
        nc.sync.dma_start(out=ascw2, in_=ascw2_d[:])
        ascb1 = const.tile([128, 1], f32)
        nc.sync.dma_start(out=ascb1, in_=ascb1_d[:])
        ascb2x2 = const.tile([128, 1], f32)
        nc.sync.dma_start(out=ascb2x2, in_=ascb2x2_d[:])
        tirb2 = const.tile([128, 1], f32)
        nc.sync.dma_start(out=tirb2, in_=tirb2_d[:])
        qm4 = const.tile([128, 1], f32)
        nc.sync.dma_start(out=qm4, in_=qm4_d[:])
        pvec = const.tile([1, 8], f32)
        nc.sync.dma_start(out=pvec, in_=params_d[:])

        b_zero = const.tile([128, 1], f32)
        nc.vector.memset(b_zero, 0.0)
        b_two = const.tile([128, 1], f32)
        nc.vector.memset(b_two, 2.0)
        b_three = const.tile([128, 1], f32)
        nc.vector.memset(b_three, 3.0)
        b_eps = const.tile([128, 1], f32)
        nc.vector.memset(b_eps, 1e-6)
        ones_row = const.tile([1, 128], f32)
        nc.vector.memset(ones_row, 1.0)

        # ---------------- collectors ----------------
        top8c = coll.tile([128, NG, 8], f32)
        redc = coll.tile([128, NG, 2], f32)

        # ---------------- main loop ----------------
        for sg in range(NSG):
            d0 = sg * SUPER
            dt_ = dpool.tile([128, SUPER, ND], bf16, tag="dt")
            nc.gpsimd.dma_start(out=dt_, in_=docT_v[:, d0:d0 + SUPER, :])

            # pairwise grams; pp layout [128, d, 360]: pp1@[0:179] pp2@[180:358]
            pp = ppool.tile([128, SUPER, 360], bf16, tag="pp")
            nc.vector.tensor_mul(pp[:, :, 0:NW2],
                                 dt_[:, :, 0:NW2], dt_[:, :, 1:ND])
            nc.vector.tensor_mul(pp[:, :, 180:180 + NW3],
                                 dt_[:, :, 0:NW3], dt_[:, :, 2:ND])
            nc.vector.memset(pp[:, :, NW2:180], 0.0)

            # D-reduction: gsum[32b+j, 0:179]=g1, [180:358]=g2 for doc 4j+b
            gsum_t = psg.tile([128, 512], f32, tag="gsum")
            gsum = gsum_t[:, 0:358]
            for dd in range(SUPER):
                j, b = dd // GROUP, dd % GROUP
                sel = selg[:, 32 * j:32 * (j + 1)]
                nc.tensor.matmul(gsum[32 * b:32 * (b + 1), :], sel,
                                 pp[:, dd, 0:358], start=True, stop=True,
                                 tile_position=(0, 32 * b),
                                 skip_group_check=True)

            # norms -> inv = rsqrt at doc resolution
            # inv layout (184-aligned regions): R2@[0:184] R3@[184:368]
            g_sb = gpool.tile([128, 358], bf16, tag="gsb")
            nc.scalar.copy(g_sb, gsum)
            t3 = gpool.tile([128, NW3], bf16, tag="t3")
            nc.vector.tensor_add(t3, g_sb[:, 0:NW3], g_sb[:, 1:NW2])
            n3in = gpool.tile([128, NW3], bf16, tag="n3in")
            nc.vector.tensor_add(n3in, t3, g_sb[:, 180:358])
            lnn = gpool.tile([128, 388], f32, tag="lnn")
            nc.scalar.activation(out=lnn[:, 0:NW2], in_=g_sb[:, 0:NW2],
                                 func=AF.Ln, bias=b_two, scale=2.0)
            nc.scalar.activation(out=lnn[:, 204:204 + NW3], in_=n3in,
                                 func=AF.Ln, bias=b_three, scale=2.0)
            inv = gpool.tile([128, 408], bf16, tag="inv")
            nc.vector.memset(inv[:, NW2:204], 0.0)
            nc.vector.memset(inv[:, 204 + NW3:408], 0.0)
            nc.scalar.activation(out=inv[:, 0:NW2], in_=lnn[:, 0:NW2],
                                 func=AF.Exp, bias=b_zero, scale=-0.5)
            nc.scalar.activation(out=inv[:, 204:204 + NW3],
                                 in_=lnn[:, 204:204 + NW3],
                                 func=AF.Exp, bias=b_zero, scale=-0.5)

            for j in range(GPS):
                gg = GPS * sg + j
                p2 = j % 2
                p4 = j % 4
                if p4 == 0:
                    # 4-group shared sims/num tile (fp16); only col 562 (the
                    # one pad lane the 179-wide segments read) needs a value
                    sev = work.tile([128, 4, 592], bf16, tag="sev")
                    nc.vector.memset(sev[:, :, 562:563], -60000.0)
                    invb_sb = work.tile([128, 4, 408], bf16, tag="invsb")
                if p2 == 0:
                    # invb spans 2 PSUM banks per 2 groups
                    invb2g_t = psb.tile([128, 1024], f32, tag="invb")
                # sims@[0:180] and num2@[180:359] accumulate in one bank
                ps_sims_t = ps.tile([128, 512], f32, tag="sims")
                ps_sims = ps_sims_t[:, 0:ND]
                ps_num2 = ps_sims_t[:, ND:ND + NW2]
                for b in range(GROUP):
                    d = GROUP * j + b
                    rows = slice(32 * b, 32 * (b + 1))
                    tp = (0, 32 * b)
                    nc.tensor.matmul(ps_sims[rows, :], qT,
                                     dt_[:, d, :], start=True, stop=True,
                                     tile_position=tp, skip_group_check=True)
                    nc.tensor.matmul(ps_num2[rows, :], qT,
                                     dt_[:, d, 0:NW2], start=True, stop=False,
                                     tile_position=tp, skip_group_check=True)
                    nc.tensor.matmul(ps_num2[rows, :], qT,
                                     dt_[:, d, 1:ND], start=False, stop=True,
                                     tile_position=tp, skip_group_check=True)
                # inv rows {32b+j} broadcast to all 32 q-rows, one matmul
                nc.tensor.matmul(invb2g_t[:, 512 * p2:512 * p2 + 408],
                                 sful[:, 128 * j:128 * (j + 1)],
                                 inv, start=True, stop=True,
                                 tile_position=(0, 0), skip_group_check=True)
                if p2 == 1:
                    # both halves written: move invb to SBUF fp16 (ACT)
                    nc.scalar.copy(
                        invb_sb[:, 2 * (p4 // 2):2 * (p4 // 2) + 2, :],
                        invb2g_t.rearrange("p (g c) -> p g c",
                                           c=512)[:, :, 0:408])

                # sims+num2 -> SBUF fp16 in one ACT copy
                nc.scalar.copy(sev[:, p4, 0:359], ps_sims_t[:, 0:359])
                nc.vector.max(top8c[:, gg, :], sev[:, p4, 0:ND])

                if p4 == 3:
                    # num3 = num2 + s[t+2], one aligned 2x DVE add for 4 grps
                    nc.vector.tensor_add(sev[:, :, 384:384 + NW3],
                                         sev[:, :, 180:180 + NW3],
                                         sev[:, :, 2:ND])
                    # scr = num*inv (all-SBUF fp16) + segmented max, 4 grp/pass
                    scr4g = work.tile([128, 4, 368], bf16, tag="scr")
                    num_v = sev[:, :, 180:588].rearrange(
                        "p g (r c) -> p g r c", c=204)[:, :, :, 0:179]
                    invs_v = invb_sb.rearrange(
                        "p g (r c) -> p g r c", c=204)[:, :, :, 0:179]
                    scr_v = scr4g.rearrange(
                        "p g (r c) -> p g r c", c=184)[:, :, :, 0:179]
                    nc.vector.tensor_mul(scr_v, num_v, invs_v)
                    # two-level max: 2x-mode pairwise fold 179 -> 90 lanes
                    # (col 89 pairs with itself), then a 1x reduce on half
                    scrH = work.tile([128, 4, 2, 90], bf16, tag="scrh")
                    nc.vector.tensor_tensor(
                        out=scrH, in0=scr_v[:, :, :, 0:90],
                        in1=scr_v[:, :, :, 89:179], op=OP.max)
                    nc.vector.reduce_max(
                        redc[:, gg - 3:gg + 1, :], scrH,
                        axis=mybir.AxisListType.X)

        # ---------------- finishing phase ----------------
        mainps.close()
        ps = ctx.enter_context(tc.tile_pool(name="psf", bufs=1, space="PSUM"))

        topv = top8c[:, :, 0:TOPK]              # [128, NG, 3]
        pmax = top8c[:, :, 0:1]                 # [128, NG, 1]

        # collector-ready stats first, so the PE runs under the pts chain
        pmax2 = fin.tile([128, NG], f32)
        nc.vector.tensor_mul(pmax2, pmax[:, :, 0], pmax[:, :, 0])
        ps_stat_t = ps.tile([128, 512], f32, tag="stat")
        ps_stat = ps_stat_t[:, 0:3 * NG]        # asc_base | mean | msq
        ps_stat2_t = ps.tile([128, 512], f32, tag="stat2")
        ps_stat2 = ps_stat2_t[:, 0:3 * NG]      # red2w | red3w | base
        ps_mlp_t = ps.tile([128, 512], f32, tag="mlp")
        ps_mlp = ps_mlp_t[:, 0:NG]              # H
        ps_base = ps_stat2[:, 2 * NG:3 * NG]
        pmax_2d = top8c[:, :, 0]                # [128, NG] strided view
        for b in range(GROUP):
            rows = slice(32 * b, 32 * (b + 1))
            tp = (32 * b, 32 * b)
            nc.tensor.matmul(ps_stat[rows, 0:NG], wq32[rows, :],
                             pmax_2d[rows, :], start=True, stop=True,
                             tile_position=tp, skip_group_check=True)
            nc.tensor.matmul(ps_stat[rows, NG:2 * NG], qmn32[rows, :],
                             pmax_2d[rows, :], start=True, stop=True,
                             tile_position=tp, skip_group_check=True)
            nc.tensor.matmul(ps_stat[rows, 2 * NG:3 * NG], qmn32[rows, :],
                             pmax2[rows, :], start=True, stop=True,
                             tile_position=tp, skip_group_check=True)
            nc.tensor.matmul(ps_stat2[rows, 0:NG], wq32[rows, :],
                             redc[rows, :, 0], start=True, stop=True,
                             tile_position=tp, skip_group_check=True)
            nc.tensor.matmul(ps_stat2[rows, NG:2 * NG], wq32[rows, :],
                             redc[rows, :, 1], start=True, stop=True,
                             tile_position=tp, skip_group_check=True)

        # pts = softmax(topv/T).topv
        exps = fin.tile([128, NG, TOPK], f32)
        nc.scalar.activation(out=exps, in_=topv, func=AF.Exp,
                             bias=b_zero, scale=1.0 / TEMP)
        den = fin.tile([128, NG], f32)
        nc.vector.reduce_sum(den, exps, axis=mybir.AxisListType.X)
        wnum = fin.tile([128, NG, TOPK], f32)
        nc.vector.tensor_mul(wnum, exps, topv)
        pnum = fin.tile([128, NG], f32)
        nc.vector.reduce_sum(pnum, wnum, axis=mybir.AxisListType.X)
        rden = fin.tile([128, NG], f32)
        nc.vector.reciprocal(rden, den)
        pts = fin.tile([128, NG], f32)
        nc.vector.tensor_mul(pts, pnum, rden)
        # padded = pts * qm (qm broadcast per partition block)
        pts_t = fin.tile([128, NG], f32)
        nc.vector.tensor_scalar(out=pts_t, in0=pts, scalar1=qm4, scalar2=None,
                                op0=OP.mult)
        for b in range(GROUP):
            rows = slice(32 * b, 32 * (b + 1))
            tp = (32 * b, 32 * b)
            nc.tensor.matmul(ps_base[rows, :], wq32[rows, :],
                             pts[rows, :], start=True, stop=True,
                             tile_position=tp, skip_group_check=True)

        # single ordered whole-bank reads (PSUM banks must not be read while
        # PE still writes other columns of the same bank)
        stat_sb = fin.tile([128, 3 * NG], f32)
        nc.scalar.copy(stat_sb, ps_stat)
        stat2_sb = fin.tile([128, 3 * NG], f32)
        nc.scalar.copy(stat2_sb, ps_stat2)
        # mx: max over q (transpose + free-dim reduce + transpose back)
        ident = const.tile([128, 128], f32)
        nc.sync.dma_start(out=ident, in_=nc.dram_tensor(
            "ident", [128, 128], f32, kind="ExternalInput")[:])
        pmax_sb = fin.tile([128, NG], f32)
        nc.vector.tensor_copy(pmax_sb, pmax_2d)
        ps_pT_t = ps.tile([128, 512], f32, tag="ptrans")
        ps_pT = ps_pT_t[:, 0:128]
        nc.tensor.transpose(ps_pT[0:NG, :], pmax_sb, ident)
        mxT = fin.tile([128, GROUP], f32)
        nc.vector.reduce_max(mxT[0:NG, :], ps_pT[0:NG, :].rearrange(
            "g (b q) -> g b q", q=32), axis=mybir.AxisListType.X)
        mxpad = fin.tile([128, 128], f32)
        nc.vector.memset(mxpad, 0.0)
        nc.vector.tensor_copy(
            mxpad[0:NG, :].rearrange("g (b o) -> g b o", o=32)[:, :, 0:1],
            mxT[0:NG, :].rearrange("g (b o) -> g b o", o=1))
        ps_mxb_t = ps.tile([128, 512], f32, tag="ptrans")
        ps_mxb = ps_mxb_t[:, 0:128]
        nc.tensor.transpose(ps_mxb[:, 0:NG], mxpad[0:NG, :], ident[0:NG, 0:NG])

        # ---- per-strip finishing: every accessed row is at base 32b ----
        F = fin.tile([128, 3 * NG], f32)    # cols: mean | mx | std, rows {32b}
        scrA = fin.tile([128, NG], f32)
        scrB = fin.tile([128, NG], f32)
        for b in range(GROUP):
            row = slice(32 * b, 32 * b + 1)
            mean = stat_sb[row, NG:2 * NG]
            msq = stat_sb[row, 2 * NG:3 * NG]
            nc.vector.tensor_copy(F[row, 0:NG], mean)
            nc.scalar.copy(F[row, NG:2 * NG], ps_mxb[row, 0:NG])
            # var = msq - mean^2 -> std = exp(0.5*ln(var + 1e-6))
            nc.vector.tensor_mul(scrA[row, :], F[row, 0:NG], F[row, 0:NG])
            nc.vector.tensor_sub(scrB[row, :], msq, scrA[row, :])
            nc.scalar.activation(out=scrA[row, :], in_=scrB[row, :],
                                 func=AF.Ln, bias=b_eps[row, :], scale=1.0)
            nc.scalar.activation(out=F[row, 2 * NG:3 * NG], in_=scrA[row, :],
                                 func=AF.Exp, bias=b_zero[row, :], scale=0.5)

        # ASC MLP via K=1 accumulation over the 3 features
        for b in range(GROUP):
            row = slice(32 * b, 32 * b + 1)
            for s in range(3):
                nc.tensor.matmul(ps_mlp[32 * b:32 * (b + 1), :],
                                 ascw1[row, 32 * s:32 * (s + 1)],
                                 F[row, s * NG:(s + 1) * NG],
                                 start=(s == 0), stop=(s == 2),
                                 tile_position=(32 * b, 32 * b),
                                 skip_group_check=True)
        Hs = fin.tile([128, NG], f32)
        nc.scalar.activation(out=Hs, in_=ps_mlp, func=AF.Relu, bias=ascb1, scale=1.0)
        ps_calsc_t = ps.tile([128, 512], f32, tag="cal")
        ps_cal = ps_calsc_t[:, 0:NG]
        ps_sc = ps_calsc_t[:, NG:NG + 8]
        for b in range(GROUP):
            nc.tensor.matmul(ps_cal[32 * b:32 * (b + 1), :],
                             ascw2[32 * b:32 * (b + 1), :],
                             Hs[32 * b:32 * (b + 1), :], start=True, stop=True,
                             tile_position=(32 * b, 32 * b), skip_group_check=True)
        # gw = softmax(mgs_logits); blend = sigmoid(asc_blend)
        ge = fin.tile([1, 3], f32)
        gden = fin.tile([1, 1], f32)
        nc.scalar.activation(out=ge, in_=pvec[0:1, 0:3], func=AF.Exp,
                             bias=b_zero[0:1, :], scale=1.0)
        nc.vector.reduce_sum(gden, ge, axis=mybir.AxisListType.X)
        grden = fin.tile([1, 1], f32)
        nc.vector.reciprocal(grden, gden)
        svec = fin.tile([1, 8], f32)
        nc.vector.memset(svec, 0.0)
        nc.vector.tensor_scalar(out=svec[0:1, 0:3], in0=ge, scalar1=grden,
                                scalar2=None, op0=OP.mult)
        be = fin.tile([1, 1], f32)
        nc.scalar.activation(out=be, in_=pvec[0:1, 3:4], func=AF.Exp,
                             bias=b_zero[0:1, :], scale=-1.0)
        nc.vector.tensor_scalar(out=be, in0=be, scalar1=1.0, scalar2=None, op0=OP.add)
        nc.vector.reciprocal(svec[0:1, 3:4], be)                       # blend
        nc.vector.tensor_scalar(out=svec[0:1, 4:5], in0=svec[0:1, 3:4],
                                scalar1=-1.0, scalar2=1.0, op0=OP.mult, op1=OP.add)
        nc.tensor.matmul(ps_sc, ones_row, svec, start=True, stop=True,
                         tile_position=(0, 0), skip_group_check=True)
        calsc_sb = fin.tile([128, NG + 8], f32)
        nc.scalar.copy(calsc_sb, ps_calsc_t[:, 0:NG + 8])
        sc = calsc_sb[:, NG:NG + 8]
        cal_sb = calsc_sb[:, 0:NG]

        # TIR MLP (row-tiled K=32); A/B in separate banks (PE-W vs ACT-R hazard)
        ps_tirA_t = ps.tile([128, 512], f32, tag="tirA")
        ps_tirA = ps_tirA_t[:, 0:NG]
        ps_tirB_t = ps.tile([128, 512], f32, tag="tirB")
        ps_tirB = ps_tirB_t[:, 0:NG]
        tir_dst = [(ps_tirA, 0), (ps_tirA, 64), (ps_tirB, 0), (ps_tirB, 64)]
        for b in range(GROUP):
            dst, o = tir_dst[b]
            nc.tensor.matmul(dst[o:o + 64, :], tirw1[32 * b:32 * (b + 1), :],
                             pts_t[32 * b:32 * (b + 1), :], start=True, stop=True,
                             tile_position=(32 * b, o), skip_group_check=True)
        HsA = fin.tile([128, NG], f32)
        HsB = fin.tile([128, NG], f32)
        nc.scalar.activation(out=HsA, in_=ps_tirA, func=AF.Relu, bias=tirb1, scale=1.0)
        nc.scalar.activation(out=HsB, in_=ps_tirB, func=AF.Relu, bias=tirb1, scale=1.0)
        ps_tv_t = ps.tile([128, 512], f32, tag="tv")
        ps_tv = ps_tv_t[:, 0:NG]
        tir_src = [(HsA, 0), (HsA, 64), (HsB, 0), (HsB, 64)]
        for b in range(GROUP):
            src_t, o = tir_src[b]
            nc.tensor.matmul(ps_tv[32 * b:32 * (b + 1), :], tirw2[o:o + 64, :],
                             src_t[o:o + 64, :], start=True, stop=True,
                             tile_position=(o, 32 * b), skip_group_check=True)

        tv_sb = fin.tile([128, NG], f32)
        nc.scalar.copy(tv_sb, ps_tv)
        # combine per strip on row {32b}:
        # total = blend*base + (1-blend)*asc_base*(1+calib)
        #         + gw0*asc_base + gw1*red2w + gw2*red3w + tirv + tir_b2
        tot = fin.tile([128, NG], f32)
        for b in range(GROUP):
            row = slice(32 * b, 32 * b + 1)
            asc_base = stat_sb[row, 0:NG]
            base = stat2_sb[row, 2 * NG:3 * NG]
            # calib1 = 1 + tanh(x + b2) = 2 - 2/(exp(2x+2b2)+1)
            nc.scalar.activation(out=scrA[row, :], in_=cal_sb[row, :],
                                 func=AF.Exp, bias=ascb2x2[row, :], scale=2.0)
            nc.vector.tensor_scalar(out=scrA[row, :], in0=scrA[row, :],
                                    scalar1=1.0, scalar2=None, op0=OP.add)
            nc.vector.reciprocal(scrB[row, :], scrA[row, :])
            nc.vector.tensor_scalar(out=scrB[row, :], in0=scrB[row, :],
                                    scalar1=-2.0, scalar2=2.0, op0=OP.mult, op1=OP.add)
            nc.vector.tensor_mul(scrB[row, :], scrB[row, :], asc_base)
            nc.vector.tensor_scalar(out=scrB[row, :], in0=scrB[row, :],
                                    scalar1=sc[row, 4:5], scalar2=None, op0=OP.mult)
            nc.vector.tensor_scalar(out=scrA[row, :], in0=base,
                                    scalar1=sc[row, 3:4], scalar2=None, op0=OP.mult)
            nc.vector.tensor_add(tot[row, :], scrA[row, :], scrB[row, :])
            nc.vector.tensor_scalar(out=scrA[row, :], in0=asc_base,
                                    scalar1=sc[row, 0:1], scalar2=None, op0=OP.mult)
            nc.vector.tensor_add(tot[row, :], tot[row, :], scrA[row, :])
            nc.vector.tensor_scalar(out=scrA[row, :], in0=stat2_sb[row, 0:NG],
                                    scalar1=sc[row, 1:2], scalar2=None, op0=OP.mult)
            nc.vector.tensor_add(tot[row, :], tot[row, :], scrA[row, :])
            nc.vector.tensor_scalar(out=scrA[row, :], in0=stat2_sb[row, NG:2 * NG],
                                    scalar1=sc[row, 2:3], scalar2=None, op0=OP.mult)
            nc.vector.tensor_add(tot[row, :], tot[row, :], scrA[row, :])
            nc.vector.tensor_add(tot[row, :], tot[row, :], tv_sb[row, :])
            nc.vector.tensor_scalar(out=tot[row, :], in0=tot[row, :],
                                    scalar1=tirb2[row, :], scalar2=None, op0=OP.add)
            nc.sync.dma_start(out=out_d[b:b + 1, :], in_=tot[row, :])

    return nc


def _legalize_single_wait(nc):
    """Walrus (this compile path) accepts at most one sync wait per
    instruction; offload extra waits onto preceding EventSemaphore
    instructions on the same engine queue."""
    from concourse import mybir
    ctr = [0]
    for bb in nc.main_func.blocks:
        il = bb.instructions
        out = []
        for inst in il:
            si = inst.sync_info
            if si is not None and len(si.on_wait) > 1:
                waits = list(si.on_wait)
                eng = nc.engines[inst.engine]
                for w in waits[:-1]:
                    ev = eng._isa(
                        nc.isa.Opcode.NEURON_ISA_TPB_OPCODE_NOP, {})
                    ev.sync_info = mybir.SyncInfo(on_wait=[w], on_update=[])
                    ctr[0] += 1
                    try:
                        nc.register_instruction(ev)
                    except Exception:
                        pass
                    out.append(ev)
                inst.sync_info = mybir.SyncInfo(on_wait=[waits[-1]],
                                                on_update=list(si.on_update))
            out.append(inst)
        bb.instructions = out
    return nc


def _host_prep(inputs, ndoc_per_core, ncores):
    """Shard + lay out inputs for the SPMD program. Returns list of in_maps."""
    q = np.asarray(inputs["query_embs"], np.float32)          # [NQ, D]
    docs = np.asarray(inputs["doc_embs"], np.float32)         # [N, ND, D]
    w = np.asarray(inputs["importance_weights"], np.float32)  # [NQ]
    qm = np.asarray(inputs["query_mask"]).astype(np.float32)  # [NQ]
    asc_w1 = np.asarray(inputs["asc_w1"], np.float32)
    asc_b1 = np.asarray(inputs["asc_b1"], np.float32)
    asc_w2 = np.asarray(inputs["asc_w2"], np.float32)
    asc_b2 = np.float32(inputs["asc_b2"])
    asc_blend = np.float32(inputs["asc_blend"])
    mgs_logits = np.asarray(inputs["mgs_logits"], np.float32)
    tir_w1 = np.asarray(inputs["tir_w1"], np.float32)
    tir_b1 = np.asarray(inputs["tir_b1"], np.float32)
    tir_w2 = np.asarray(inputs["tir_w2"], np.float32)
    tir_b2 = np.float32(inputs["tir_b2"])

    wq = (w * qm).astype(np.float32)
    nvalid = float(qm.sum())
    frac = nvalid / NQ

    # frac folded into ASC layer-1 bias; feats order = [mean, mx, std]
    b1p = asc_b1 + frac * asc_w1[3, :]
    w1p = asc_w1[:3, :]                                       # [3, 32]

    GPS = SUPER // GROUP
    qT = np.ascontiguousarray(q.T).astype(BF)                 # [128, 32]
    selg = np.zeros((128, 32 * GPS), BF)
    for j in range(GPS):
        selg[:, 32 * j + j] = 1
    # sful: S_j[32b+j, 128j + 32b+q] = 1 (bcast inv row 32b+j -> rows 32b+q)
    sful = np.zeros((128, 128 * GPS), BF)
    for j in range(GPS):
        for b in range(GROUP):
            sful[32 * b + j, 128 * j + 32 * b:128 * j + 32 * (b + 1)] = 1
    wq32 = np.zeros((128, 32), np.float32)
    wq32[:, 0] = np.tile(wq, 4)
    qmn32 = np.zeros((128, 32), np.float32)
    qmn32[:, 0] = np.tile(qm / max(nvalid, 1e-9), 4)
    tirw1 = np.tile(tir_w1, (4, 1)).astype(np.float32)        # [128, 64]
    tirw2 = np.zeros((128, 32), np.float32)
    tirw2[:, 0] = np.tile(tir_w2, 2)
    tirb1 = np.tile(tir_b1, 2).reshape(128, 1).astype(np.float32)
    ascw1 = np.zeros((128, 96), np.float32)
    for b in range(4):
        for s in range(3):
            ascw1[32 * b, 32 * s:32 * (s + 1)] = w1p[s, :]
    ascw2 = np.zeros((128, 32), np.float32)
    ascw2[:, 0] = np.tile(asc_w2, 4)
    ascb1 = np.tile(b1p, 4).reshape(128, 1).astype(np.float32)
    ascb2x2 = np.full((128, 1), 2.0 * asc_b2, np.float32)
    tirb2 = np.full((128, 1), tir_b2, np.float32)
    qm4 = np.tile(qm, 4).reshape(128, 1).astype(np.float32)
    params = np.zeros((1, 8), np.float32)
    params[0, 0:3] = mgs_logits
    params[0, 3] = asc_blend
    ident = np.eye(128, dtype=np.float32)

    shared = dict(qT=qT, selg=selg, sful=sful, wq32=wq32, qmn32=qmn32,
                  tirw1=tirw1, tirw2=tirw2, tirb1=tirb1, ascw1=ascw1,
                  ascw2=ascw2, ascb1=ascb1, ascb2x2=ascb2x2, tirb2=tirb2,
                  qm4=qm4, params=params, ident=ident)

    in_maps = []
    for c in range(ncores):
        sl = docs[c * ndoc_per_core:(c + 1) * ndoc_per_core]   # [ndoc, ND, D]
        dT = np.ascontiguousarray(sl.transpose(2, 0, 1)).astype(BF)
        m = dict(shared)
        m["docT"] = dT.reshape(128, ndoc_per_core * ND)
        in_maps.append(m)
    return in_maps


def _numpy_fallback(inputs):
    """Full-precision numpy implementation (only for non-all-ones masks)."""
    q = np.asarray(inputs["query_embs"], np.float64)
    docs = np.asarray(inputs["doc_embs"], np.float64)
    w = np.asarray(inputs["importance_weights"], np.float64)
    qm_b = np.asarray(inputs["query_mask"]).astype(bool)
    dm_b = np.asarray(inputs["doc_mask"]).astype(bool)
    NEG = -1e9
    qm = qm_b.astype(np.float64)
    wq = w * qm
    sims = np.einsum("qd,ntd->nqt", q, docs)
    sims = np.where(dm_b[:, None, :], sims, NEG)
    topv = -np.sort(-sims, axis=-1)[:, :, :TOPK]
    e = np.exp((topv - topv.max(-1, keepdims=True)) / TEMP)
    soft = e / e.sum(-1, keepdims=True)
    pts = (soft * topv).sum(-1)
    base = pts @ wq
    pmax = sims.max(-1)
    asc_base = pmax @ wq
    nvalid = qm.sum()
    mean = (pmax * qm).sum(-1) / nvalid
    mx = np.where(qm_b, pmax, NEG).max(-1)
    std = np.sqrt((((pmax - mean[:, None]) ** 2) * qm).sum(-1) / nvalid + 1e-6)
    frac = np.full_like(mean, nvalid / NQ)
    feats = np.stack([mean, mx, std, frac], -1)
    h = np.maximum(feats @ np.asarray(inputs["asc_w1"], np.float64)
                   + np.asarray(inputs["asc_b1"], np.float64), 0)
    calib = np.tanh(h @ np.asarray(inputs["asc_w2"], np.float64)
                    + float(inputs["asc_b2"]))
    asc_score = asc_base * (1.0 + calib)
    blend = 1 / (1 + np.exp(-float(inputs["asc_blend"])))
    total = blend * base + (1 - blend) * asc_score
    gl = np.asarray(inputs["mgs_logits"], np.float64)
    gw = np.exp(gl - gl.max()); gw /= gw.sum()
    dmf = dm_b.astype(np.float64)
    for k in range(1, MAXK + 1):
        if k == 1:
            dk, mk = docs, dm_b
        else:
            nw = ND - k + 1
            s = sum(docs[:, i:i + nw] for i in range(k)) / k
            dk = s / np.sqrt((s * s).sum(-1, keepdims=True) + 1e-12)
            mkf = dmf[:, 0:nw].copy()
            for i in range(1, k):
                mkf = mkf * dmf[:, i:i + nw]
            mk = mkf > 0.5
        sk = np.einsum("qd,nwd->nqw", q, dk)
        sk = np.where(mk[:, None, :], sk, NEG)
        total = total + gw[k - 1] * (sk.max(-1) @ wq)
    padded = pts * qm
    hres = np.maximum(padded @ np.asarray(inputs["tir_w1"], np.float64)
                      + np.asarray(inputs["tir_b1"], np.float64), 0)
    total = total + hres @ np.asarray(inputs["tir_w2"], np.float64) + float(inputs["tir_b2"])
    return total.astype(np.float32)


def kernel(**inputs):
    qm = np.asarray(inputs["query_mask"]).astype(bool)
    dm = np.asarray(inputs["doc_mask"]).astype(bool)
    if not (qm.all() and dm.all()):
        return _numpy_fallback(inputs)

    from concourse.bass_utils import run_bass_kernel_spmd

    key = ("prog", NDOC)
    if key not in _CACHE:
        from concourse import mybir
        nc = _legalize_single_wait(_build_program(NDOC))
        # populate .instr bytes for extended-inst ISA subclasses
        # (tensor_tensor_reduce); without this walrus fails with
        # "ISA wrong length".
        mybir.codegen_inst_isa_subclasses(nc)
        _CACHE[key] = nc
    nc = _CACHE[key]

    in_maps = _host_prep(inputs, NDOC, NCORES)
    res = run_bass_kernel_spmd(nc, in_maps, list(range(NCORES)))
    out = np.empty((N,), np.float32)
    for c in range(NCORES):
        grid = res.results[c]["out"]            # [4, NG] -> doc = 4*g + b
        out[c * NDOC:(c + 1) * NDOC] = grid.T.reshape(-1)
    return out


if __name__ == "__main__":
    # quick CoreSim correctness check on a reduced doc count
    from concourse.bass_interp import CoreSim

    nd = int(os.environ.get("SIM_NDOC", "32"))
    rng = np.random.default_rng(0)

    def l2n(x):
        return x / np.sqrt((x * x).sum(-1, keepdims=True) + 1e-12)

    inputs = {
        "query_embs": l2n(rng.standard_normal((NQ, D))).astype(np.float32),
        "doc_embs": l2n(rng.standard_normal((nd, ND, D))).astype(np.float32),
        "importance_weights": rng.random(NQ).astype(np.float32),
        "query_mask": np.ones(NQ, bool),
        "doc_mask": np.ones((nd, ND), bool),
        "asc_w1": (rng.standard_normal((4, 32)) * 0.1).astype(np.float32),
        "asc_b1": np.zeros(32, np.float32),
        "asc_w2": (rng.standard_normal(32) * 0.1).astype(np.float32),
        "asc_b2": np.float32(0.0),
        "asc_blend": np.float32(0.5),
        "mgs_logits": (rng.standard_normal(3) * 0.1).astype(np.float32),
        "tir_w1": (rng.standard_normal((NQ, 64)) * 0.1).astype(np.float32),
        "tir_b1": np.zeros(64, np.float32),
        "tir_w2": (rng.standard_normal(64) * 0.1).astype(np.float32),
        "tir_b2": np.float32(0.0),
    }
    globals()["N"] = nd  # shrink problem for sim
    expected = _numpy_fallback(inputs)

    nc = _build_program(nd)
    in_maps = _host_prep(inputs, nd, 1)
    sim = CoreSim(nc)
    for k, v in in_maps[0].items():
        sim.tensor(k)[:] = v
    sim.simulate()
    grid = np.array(sim.tensor("out"))
    got = grid.T.reshape(-1)
    err = np.abs(got - expected)
    rel = err.max() / np.abs(expected).max()
    print("expected[:8]:", expected[:8])
    print("got[:8]     :", got[:8])
    print("max abs err:", err.max(), " rel:", rel)
    print("sim time (ns):", sim.time)
